# revision 64
# baseline (speedup 1.0000x reference)
"""Masked dot-product attention (B=16, Q=K=2048, D=64) on 8 Trainium2 cores.

out = softmax(Q K^T / sqrt(64) + mask(valid_lens)) V, reproducing
reference.py's masked_softmax to ~7e-3 relative absmax (fp16 matmuls +
a bit-trick exp on two of the three exp engines, see below).

Sharding / load balance
-----------------------
Work units are (batch, 512-wide q-block): 64 units whose cost is
nk(b) = ceil(valid_len[b]/128) k-tiles. Units are sorted by nk descending and
dealt round-robin into 8 slots x 8 cores, so every core runs the *same*
static SPMD program (slot j processes NK_j = max-nk-of-its-rank-group
k-tiles) while the host packs each core's own data. Per-core inputs arrive
as packed [128, *] fp16 buffers per slot: [Q^T dup | K^T half-packed] and
[V_aug] (see _xw). Q|K and V ride separate DMAs, with V transfers trailing
so the early (ramp-limited) DMA bandwidth all feeds the S-matmul stream.

Device pipeline (inputs fp16; PSUM accumulates fp32)
----------------------------------------------------
  PE : S^T[128k, 512q] per k-tile = matmul(lhsT=K^T-tile, rhs=Q^T),
       contraction d=64, alternating k-tiles on PE row groups 0-63/64-127
  exp: P = exp(S^T/8) over the 3-bank PSUM group, split BY QUERY COLUMN
       across the two engines that can read PSUM (so each softmax row sees
       exactly one approximation and any per-engine scale cancels in the
       row's own denominator):
         - ACT cols [0:ACOL) of each 512 block: table-exact EXP ACTIVATE
         - DVE cols [ACOL:512): product bit-trick (below)
       (Pool/GPSIMD can't read PSUM and its tensor ops measured ~6-18x
       slower than modeled, so it only runs DMA issues here.)
  PE : O^T_aug[65, 512q] += matmul(lhsT=V_aug-tile[128,65], rhs=P-slice)

Product bit-trick exp (2 DVE-cycles/elem vs 1 ACT-cycle, but on otherwise
idle engines): exp(x/8) = 2^t (t = x*log2e/8) ~= f16bits(i) * f16bits(i-512)
where i = int16(x*64*log2e + BA). Each factor is a half-exponent Schraudolph
approximant 2^(t/2+c)*r(frac) with ripple r; the two factors sit exactly half
a mantissa-period apart, so the product's log-ripple cancels the odd
harmonics: +-1.5% max element error (vs +-6% for one trick). Empirically
7.3e-3 relative absmax end-to-end on the real inputs (tolerance 2e-2).
The int16 affine runs on fp32 PSUM directly; i-512 is an exact int
subtract; both factors multiply fp16->fp16 into the P tile.

O-matmuls lag one group behind S so the PE queue never head-of-line
blocks on exp. V_aug = [V | 1] with rows >= valid_len zeroed by the host
(exact masking, free denominator in row 64). A burst of dependency-free
tiny warm-up matmuls (reading the framework's pre-barrier const tile)
keeps the PE busy from t=0 until the first QK transfer lands, opening the
HAM clock gate to 2.4 GHz; the first COLD_GROUPS groups' O-matmuls defer
on top of that.

Close/division epilogue (all spread via a microtask FIFO)
---------------------------------------------------------
Per-unit close work is queued as thunks and pumped <=2-3 per group
boundary so no close ever inserts a multi-us bubble into an engine's exp
cadence: the [65,512] PSUM->SBUF numerator+denominator copy (DVE,
fp32->fp16) goes in 3 column chunks, the denominator-row hop rides the
Pool DMA queue. Reciprocals run on ACT as r = exp(-ln(d)) (Exp and Ln
share one pinned activation table set - no table switch): positions 0-5
as one batched [6,512] pair, broadcast across the 64 d-partitions via a
DRAM-bounce DMA (out-hop Sync / in-hop Pool overlap their SWDGE waits),
then fp16-multiplied and DMA'd out on alternating queues, all under the
stream. Position 6 runs the same chain solo with its two ACT ops in
separate groups and a ones-column PE broadcast. Only the final position
divides on the tail: Ln straight off its live PSUM denominator row, PE
broadcast, multiply, out. Output is fp16 [slot, 64, 512]; the host casts
and transposes O^T -> O while unsharding (numerators/denominators peak
~7.6e3 here, 8.6x under fp16 max).
"""

import sys

if "/opt/trn_rl_repo" not in sys.path:
    sys.path.insert(0, "/opt/trn_rl_repo")

import numpy as np

import bass_rust as _bass_rust
import concourse.bass as bass
import concourse.mybir as mybir
import concourse.tile as tile
from concourse import bacc
from concourse.bass_utils import run_bass_kernel_spmd
from concourse.hw_specs import get_activation_tables

B, Q, KLEN, D = 16, 2048, 2048, 64
QB = 512                      # q-block width per work unit
NCORES = 8
NSLOTS = (B * (Q // QB)) // NCORES   # 8 slots per core
KT = 128                      # k-tile height
GK = 3                        # k-tiles per exp group (3 PSUM banks)
NWARM = 40                    # dependency-free tiny PE warm-up matmuls
                              # (~28ns each, opening the HAM clock gate;
                              # more would risk queueing ahead of the
                              # first real S-matmul on the in-order PE)
COLD_GROUPS = 4               # exp groups whose O-matmuls defer to warm PE
O_LAG = 1                     # groups the O-matmuls trail the exp stream by
F32 = mybir.dt.float32
F16 = mybir.dt.float16
I16 = mybir.dt.int16
NPF16 = np.float16
AF = mybir.ActivationFunctionType
ALU = mybir.AluOpType

# --- exp column split (per 512-wide block) ---
# Pool/GPSIMD cannot read PSUM and its int16 tensor ops run ~18x slower
# than the DVE (software Q7 path, measured 14.8ns/elem), so the trick
# share lives entirely on the DVE: affine PSUM->int16, int subtract,
# fp16 multiply, ~3.2ns/elem total vs ACT's exact-table 0.83ns/elem.
ACOL = 404                    # ACT table-exact exp
DVCOL = QB - ACOL             # DVE product bit-trick
TRICK = DVCOL
LOG2E = 1.4426950408889634
TS_SCALE = 64.0 * LOG2E       # i = round(S*TS_SCALE + TS_BIAS)
TS_BIAS = 15817.1             # 15*1024 - Ca, Ca = -457.1 (centering: cosmetic)

LAST_RESULTS = None           # BassKernelResults of the most recent run

_cache: dict = {}


class _Bacc(bacc.Bacc):
    """Bacc whose activation-table fixpoint keeps every ACTIVATE on one
    set: Exp and Ln both live in natural_log_exp_and_others, but the
    default selector binds Exp to exp_and_others and then pays two ~1.3us
    ACT_TABLE_LOAD switches around the division's Ln. Restricting Exp/Ln
    to the combined set yields a single load."""

    def insert_act_table_loads(self):
        has_activation = any(
            isinstance(i, mybir.InstActivation)
            for b in self.main_func.blocks
            for i in b.instructions
        )
        if not has_activation:
            return
        tables = []
        for name, fns in get_activation_tables(self.m.arch).items():
            if name != "natural_log_exp_and_others":
                fns = fns - {AF.Exp, AF.Ln}
            tables.append((name, fns))
        _bass_rust.insert_act_table_loads(self, tables)


def _schedule(valid_lens):
    """Static work schedule from valid_lens (host-known at call time)."""
    nk = [max(1, -(-int(v) // KT)) for v in valid_lens]
    units = [(b, qb) for b in range(B) for qb in range(Q // QB)]
    units.sort(key=lambda u: (-nk[u[0]], u))
    slots_nk = [nk[units[NCORES * j][0]] for j in range(NSLOTS)]
    assign = [[units[NCORES * j + c] for j in range(NSLOTS)] for c in range(NCORES)]
    offs = np.concatenate([[0], np.cumsum(slots_nk)]).tolist()
    return nk, slots_nk, offs, assign


def _order(slots_nk):
    """Processing order: ascending through position 4 (small transfers
    land under the ramping DMA, so the early exp stream never outruns
    supply - the 2nd-biggest slot at position 3 measured 2.5-3.3us of
    early stalls), then 2nd-biggest / 3rd-biggest, biggest last so its
    transfer has the whole stream to arrive and only its own division
    sits on the tail."""
    asc = sorted(range(NSLOTS), key=lambda j: (slots_nk[j], j))
    return [asc[0], asc[1], asc[2], asc[3], asc[4], asc[6], asc[5], asc[7]]


def _xw(w):
    """Per-slot packed widths: [Q^T dup | K^T half-packed] + [V_aug].

    K^T tiles alternate between partition halves (even k-tile i in rows
    0-63 at column block i//2, odd in rows 64-127) - half the K DMA bytes
    of a full duplication while keeping the PE row-group pairing; only
    the small Q^T block is duplicated. (Shipping Q once + an on-chip dup
    DMA measured WORSE: the extra issue serialization and the dup
    dependency on odd k-tiles cost more than the 65KB/slot of wire.)"""
    kk = (w + 1) // 2
    return QB + kk * KT, w * 65


def _build(slots_nk, offs):
    """Build + compile the single SPMD program for the given slot profile."""
    order = _order(slots_nk)
    xw = [sum(_xw(w)) for w in slots_nk]
    xoffs = np.concatenate([[0], np.cumsum([xw[j] for j in order])]).tolist()
    rbatch = [0, 1, 2, 3, 4, 5]   # one division batch: Ln/Exp cost is per
                                  # batch (free-dim cycles), not per unit

    nc = _Bacc()
    data_d = nc.dram_tensor("data", [2 * D, xoffs[-1]], F16,
                            kind="ExternalInput").ap()
    # fp16 output: the host casts to fp32 while unsharding; numerators and
    # denominators peak ~7.6e3 on this distribution, 8.6x under fp16 max
    out_d = nc.dram_tensor("out", [NSLOTS, D, QB], F16, kind="ExternalOutput").ap()

    with tile.TileContext(nc) as tc:
        with (
            tc.tile_pool(name="spool", bufs=8) as spool,
            tc.tile_pool(name="vpool", bufs=8) as vpool,
            tc.tile_pool(name="ppool", bufs=8) as ppool,
            tc.tile_pool(name="tpool", bufs=3) as tpool,
            tc.tile_pool(name="epool", bufs=3) as epool,
            tc.tile_pool(name="gpool", bufs=1) as gpool,
            tc.tile_pool(name="opool", bufs=8) as opool,
            tc.tile_pool(name="dpool", bufs=2, space="DRAM") as dpool,
            tc.tile_pool(name="psum_s", bufs=2, space="PSUM") as psum_s,
            tc.tile_pool(name="psum_o", bufs=2, space="PSUM") as psum_o,
        ):
            dn_tile = gpool.tile([len(rbatch), QB], F16, name="dn", tag="dn")
            ones_sb = gpool.tile([1, D], F16, name="ones", tag="ones")
            nc.vector.memset(ones_sb, 1.0)
            # tiny first DMA warms the Sync HWDGE queue (its ~1.3us
            # activation latency) before the first real input transfer
            wdma = gpool.tile([1, D], F16, name="wdma", tag="wdma")
            nc.sync.dma_start(out=wdma, in_=data_d[0:1, 0:D])

            # PE warm-up: dependency-free matmuls from t=0 keep the PE busy
            # through the HAM activity window so the clock-gate opens to
            # 2.4 GHz before (or soon after) the first real matmul. They
            # read the framework's const tile (memset BEFORE the preamble
            # barrier) so the first one issues the moment the PE queue
            # reaches our block - no wait on our own memsets.
            cbf = nc.const_aps.aps[(mybir.dt.bfloat16, 1.0)]
            warm_ps = psum_o.tile([65, QB], F32, name="warm_ps", tag="po")
            for _ in range(NWARM):
                nc.tensor.matmul(warm_ps[0:1, 0:1], lhsT=cbf, rhs=cbf,
                                 start=True, stop=True)

            o_tiles = {}
            # Per-unit close work (PSUM->SBUF copy chunks, denominator-row
            # hops, the division batch, final multiplies) is spread across
            # group boundaries via this FIFO of thunks: at most MICRO_PUMP
            # of them run per boundary, so no single close ever inserts a
            # multi-us bubble into the DVE/ACT exp cadence.
            microq = []      # (min_gi, fn): fn emits once cur group >= min_gi
            cur = {"gi": 0}
            MICRO_PUMP = 2

            def pump_micro(n=MICRO_PUMP, drain=False):
                ran = 0
                i = 0
                while i < len(microq) and (ran < n or drain):
                    if drain or microq[i][0] <= cur["gi"]:
                        microq.pop(i)[1]()
                        ran += 1
                    else:
                        i += 1

            rb_holder = {}

            def div_stage1():
                # r = exp(-ln(d)) on ACT, one batched [6,512] op pair
                # (free-dim cycles are paid once per batch); both functions
                # share the pinned activation table set, so no table
                # switch. Broadcast across the 64 d-partitions via a DRAM
                # bounce (DRAM is flat, so the read-back replicates with a
                # stride-0 leading dim). Both hops ride the Pool queue so
                # they never block Sync input issues.
                n = len(rbatch)
                lnd = epool.tile([n, QB], F32, tag="lnd")
                nc.scalar.activation(lnd, dn_tile, AF.Ln)
                r_sb = epool.tile([n, QB], F16, tag="r")
                nc.scalar.activation(r_sb, lnd, AF.Exp, scale=-1.0)
                scratch = dpool.tile([n, QB], F16, tag="scr")
                # out-hop on Sync (idle mid-stream): overlapping the two
                # hops across queues shaves the serialized SWDGE waits
                nc.sync.dma_start(out=scratch, in_=r_sb)
                rb_sb = epool.tile([D, n, QB], F16, tag="rb")
                bcast_src = bass.AP(
                    tensor=scratch.tensor,
                    offset=scratch.offset,
                    ap=[[0, D]] + [list(a) for a in scratch.ap],
                )
                nc.gpsimd.dma_start(out=rb_sb, in_=bcast_src)
                rb_holder["rb"] = rb_sb

            def stage2_one(ui, jj):
                def f():
                    oo_sb = opool.tile([D, QB], F16, tag="oo")
                    nc.vector.tensor_mul(oo_sb, o_tiles[jj],
                                         rb_holder["rb"][:, ui, :])
                    # alternate queues: 8 back-to-back issues on one queue
                    # serialize ~5us of tail otherwise
                    eng = nc.sync if jj % 2 == 0 else nc.gpsimd
                    eng.dma_start(out=out_d[order[jj]], in_=oo_sb)
                return f

            slot_ctx = {}

            def open_qk(jidx):
                j = order[jidx]
                w = slots_nk[j]
                wqk, wv = _xw(w)
                qk_sb = spool.tile([2 * D, wqk], F16, tag="xqk")
                # early QK issues alternate Sync / ACT-HWDGE: the ACT queue
                # is idle until its first exp (~10.7us), so pairing the
                # issues halves the ~0.6us-each serialization right where
                # the stream is input-supply-limited
                eng = nc.scalar if jidx in (1, 3) else nc.sync
                eng.dma_start(
                    out=qk_sb,
                    in_=data_d[:, xoffs[jidx]:xoffs[jidx] + wqk])
                po = psum_o.tile([65, QB], F32, tag="po")
                slot_ctx[jidx] = [qk_sb, None, po, w]

            def open_v(jidx):
                j = order[jidx]
                w = slots_nk[j]
                wqk, wv = _xw(w)
                xv_sb = vpool.tile([2 * D, wv], F16, tag="xv")
                nc.sync.dma_start(
                    out=xv_sb,
                    in_=data_d[:, xoffs[jidx] + wqk:xoffs[jidx] + wqk + wv])
                slot_ctx[jidx][1] = xv_sb

            def close_slot(jidx, last=False):
                _, _, po, _ = slot_ctx[jidx]
                if jidx == NSLOTS - 1:
                    # final position: the whole division chain runs inline
                    # at the close - Ln straight off the PSUM denominator
                    # row in parallel with the DVE numerator copy, then
                    # r = exp(-ln(d)), ones-column PE broadcast, multiply,
                    # out. All tiles are dedicated (gpool) so no pool-
                    # rotation dependency can delay this tail.
                    lnd = gpool.tile([1, QB], F32, name=f"lnd{jidx}",
                                     tag=f"lnd{jidx}")
                    nc.scalar.activation(lnd, po[64:65, :], AF.Ln)
                    oa_sb = gpool.tile([D, QB], F16, name=f"oa{jidx}",
                                       tag=f"oa{jidx}")
                    nc.vector.tensor_copy(oa_sb, po[0:64, :])
                    r16 = gpool.tile([1, QB], F16, name=f"r16{jidx}",
                                     tag=f"r16{jidx}")
                    nc.scalar.activation(r16, lnd, AF.Exp, scale=-1.0)
                    # broadcast target from the psum_o pool: the rotation
                    # lands it on this unit's own accumulator bank, whose
                    # readers are this chain's own upstream
                    bc = psum_o.tile([65, QB], F32, tag="po")
                    nc.tensor.matmul(bc[0:D, :], lhsT=ones_sb,
                                     rhs=r16, start=True, stop=True)
                    oo_sb = opool.tile([D, QB], F16, tag="oo")
                    nc.vector.tensor_mul(oo_sb, oa_sb, bc[0:D, :])
                    nc.sync.dma_start(out=out_d[order[jidx]], in_=oo_sb)
                    return
                # the [65,512] PSUM->SBUF copy (fp32->fp16 in the op, DVE:
                # ACT is the exp-cadence-critical engine) moves numerator +
                # denominator together and frees the PSUM bank. For units
                # followed by a long slot it's split into column chunks and
                # pumped one per group boundary so the DVE exp chain never
                # takes a ~700ns hit in one group; the first two (short)
                # positions copy whole - their PSUM bank is re-needed a
                # single group later.
                oa_sb = gpool.tile([65, QB], F16, name=f"oa{jidx}",
                                   tag=f"oa{jidx}")
                # the batch-closing position copies whole so its
                # denominator row is ready before the batch Ln needs it
                nchunks = 1 if (jidx < 2 or jidx == rbatch[-1]) else 3
                step = -(-QB // nchunks)
                for c0 in range(0, QB, step):
                    c1 = min(QB, c0 + step)
                    if nchunks == 1:
                        nc.vector.tensor_copy(oa_sb[:, c0:c1], po[:, c0:c1])
                    else:
                        microq.append(
                            (0, lambda a=c0, b=c1:
                             nc.vector.tensor_copy(oa_sb[:, a:b],
                                                   po[:, a:b])))
                o_tiles[jidx] = oa_sb[0:D, :]
                if jidx in rbatch:
                    ri = rbatch.index(jidx)
                    if jidx == rbatch[-1]:
                        # immediate hop + near-term division: the whole
                        # chain (Ln/Exp/bounce/mults/out-DMAs) has ~5us of
                        # queue+DMA latency and must finish under the
                        # stream, not after it
                        nc.gpsimd.dma_start(out=dn_tile[ri:ri + 1, :],
                                            in_=oa_sb[64:65, :])
                        microq.append((cur["gi"] + 2, div_stage1))
                        for ui, jj in enumerate(rbatch):
                            microq.append((cur["gi"] + 4 + ui // 2,
                                           stage2_one(ui, jj)))
                    else:
                        microq.append(
                            (0, lambda: nc.gpsimd.dma_start(
                                out=dn_tile[ri:ri + 1, :],
                                in_=oa_sb[64:65, :])))
                else:
                    # second-to-last position: solo division via microq -
                    # its two ACT ops land in separate groups (each mostly
                    # absorbed by per-group ACT slack) instead of a 1.3us
                    # block that stalls the exp cadence; the broadcast is a
                    # single PE ones-matmul once r16 is ready.
                    g0 = cur["gi"]
                    lnd6 = gpool.tile([1, QB], F32, name="lnd6", tag="lnd6")
                    r166 = gpool.tile([1, QB], F16, name="r166", tag="r166")
                    microq.append(
                        (g0 + 2, lambda: nc.scalar.activation(
                            lnd6, oa_sb[64:65, :], AF.Ln)))
                    microq.append(
                        (g0 + 3, lambda: nc.scalar.activation(
                            r166, lnd6, AF.Exp, scale=-1.0)))

                    def fin6():
                        bc = psum_o.tile([65, QB], F32, tag="po")
                        nc.tensor.matmul(bc[0:D, :], lhsT=ones_sb,
                                         rhs=r166, start=True, stop=True)
                        oo_sb = opool.tile([D, QB], F16, tag="oo")
                        nc.vector.tensor_mul(oo_sb, o_tiles[jidx],
                                             bc[0:D, :])
                        nc.gpsimd.dma_start(out=out_d[order[jidx]],
                                            in_=oo_sb)
                    microq.append((g0 + 4, fin6))

            # all input DMAs are issued up front (spool/vpool hold one
            # buffer per slot, so no rotation waits). Q|K transfers lead
            # and V transfers trail two slots behind.
            open_qk(0)
            open_qk(1)
            open_qk(2)
            for jidx in range(3, NSLOTS):
                open_v(jidx - 3)
                open_qk(jidx)
            for jidx in range(NSLOTS - 3, NSLOTS):
                open_v(jidx)

            # flat k-tile schedule: exp groups are GK consecutive k-tiles
            # REGARDLESS of slot boundaries, so every exp op but the last
            # runs at the full N=1536 and slot transitions produce no
            # short-group hiccups
            flat = []
            for jidx, j in enumerate(order):
                w = slots_nk[j]
                for ki in range(w):
                    flat.append((jidx, ki, ki == w - 1))
            # group 0 covers slot 0 alone so the first exp gates only on
            # the first (smallest) QK transfer
            w0 = min(slots_nk[order[0]], GK)
            fgroups = [flat[:w0]] + [flat[i:i + GK]
                                     for i in range(w0, len(flat), GK)]

            def run_group(items, last=False):
                for pj, ki, closes, ph, p_prev in items:
                    _, pxv, ppo, pw = slot_ctx[pj]
                    pva = pxv.rearrange("p (w c) -> p w c", c=65)
                    nc.tensor.matmul(
                        ppo,
                        lhsT=pva[:, ki, :],
                        rhs=p_prev[:, ph * QB:(ph + 1) * QB],
                        start=(ki == 0), stop=(ki == pw - 1),
                    )
                    if closes:
                        close_slot(pj, last=last)

            def emit_exp(ps, p_sb, g, force_act=False):
                """Two-engine exp over a [128, g*512] PSUM group, split by
                query column (see module docstring). force_act runs the
                whole group on ACT - used for the first and last groups so
                the pipeline's ends don't wait on the DVE chain."""
                ww = g * QB
                if force_act:
                    nc.scalar.activation(p_sb[:, :ww], ps[:, :ww],
                                         AF.Exp, scale=0.125)
                    return
                ps3 = ps[:, :ww].rearrange("p (g q) -> p g q", q=QB)
                p3 = p_sb[:, :ww].rearrange("p (g q) -> p g q", q=QB)
                # ACT: table-exact exp on its column share
                nc.scalar.activation(
                    p3[:, :, 0:ACOL], ps3[:, :, 0:ACOL], AF.Exp, scale=0.125)
                # DVE: product bit-trick on its column share
                ia = tpool.tile([128, GK * TRICK], I16, tag="ia")
                ib = tpool.tile([128, GK * TRICK], I16, tag="ib")
                ia3 = ia[:, :g * TRICK].rearrange("p (g q) -> p g q", q=TRICK)
                ib3 = ib[:, :g * TRICK].rearrange("p (g q) -> p g q", q=TRICK)
                iaf = ia[:, :g * TRICK].bitcast(F16).rearrange(
                    "p (g q) -> p g q", q=TRICK)
                ibf = ib[:, :g * TRICK].bitcast(F16).rearrange(
                    "p (g q) -> p g q", q=TRICK)
                nc.vector.tensor_scalar(
                    ia3, ps3[:, :, ACOL:],
                    TS_SCALE, TS_BIAS, ALU.mult, ALU.add)
                nc.vector.tensor_scalar(ib3, ia3, 512.0, None, ALU.subtract)
                nc.vector.tensor_mul(p3[:, :, ACOL:], iaf, ibf)

            # While the PE HAM clock-gate is still cold (1.2 GHz), S
            # matmuls alone just keep pace with the exp stream, but S+O
            # would stall it: O-groups from the first COLD_GROUPS groups
            # defer entirely, then the backlog drains two-per-group down
            # to a steady O_LAG-group lag.
            ngroups = len(fgroups)
            oqueue = []   # deferred O-group batches, oldest first
            for gi, grp in enumerate(fgroups):
                cur["gi"] = gi
                g = len(grp)
                ps = psum_s.tile([128, GK * QB], F32, tag="ps")
                for i, (jidx, ki, _) in enumerate(grp):
                    qk_sb = slot_ctx[jidx][0]
                    qt_sb = qk_sb[:, 0:QB]
                    kt_sb = qk_sb[:, QB:]
                    rg = (ki % 2) * D   # row-group half = k-tile parity
                    nc.tensor.matmul(
                        ps[:, i * QB:(i + 1) * QB],
                        lhsT=kt_sb[rg:rg + D, (ki // 2) * KT:
                                   (ki // 2 + 1) * KT],
                        rhs=qt_sb[rg:rg + D, :],
                        start=True, stop=True,
                        tile_position=(rg, 0),
                    )
                if gi > COLD_GROUPS:
                    drains = (2 if len(oqueue) > O_LAG + 1 else
                              1 if len(oqueue) > O_LAG else 0)
                    for _ in range(drains):
                        run_group(oqueue.pop(0))
                p_sb = ppool.tile([128, GK * QB], F16, tag="p")
                emit_exp(ps, p_sb, g, force_act=(gi == 0))
                oqueue.append([(jidx, ki, closes, i, p_sb)
                               for i, (jidx, ki, closes) in enumerate(grp)])
                # pump harder near the end so no close work spills past
                # the last O-matmuls into the tail
                pump_micro(3 if gi >= ngroups - 6 else MICRO_PUMP)
            cur["gi"] = ngroups + 10
            pump_micro(drain=True)
            while oqueue:
                run_group(oqueue.pop(0), last=(len(oqueue) == 1))

    nc.compile()
    return nc


def _pack(queries, keys, values, valid_lens, slots_nk, offs, assign):
    order = _order(slots_nk)
    xw = [sum(_xw(w)) for w in slots_nk]
    tot = sum(xw)
    data = np.zeros((NCORES, 2 * D, tot), NPF16)
    for c in range(NCORES):
        x0 = 0
        for p, j in enumerate(order):
            b, qb = assign[c][j]
            w = slots_nk[j]
            wqk, wv = _xw(w)
            vl = int(valid_lens[b])
            blk = data[c, :, x0:x0 + xw[j]]
            qt = queries[b, qb * QB:(qb + 1) * QB, :].T      # [D, QB]
            blk[:D, 0:QB] = qt
            blk[D:, 0:QB] = qt
            # K^T tiles alternate partition halves: even k-tile i in rows
            # 0-63, odd in rows 64-127, both at column block i//2
            for i in range(w):
                half = (i % 2) * D
                c0 = QB + (i // 2) * KT
                blk[half:half + D, c0:c0 + KT] = (
                    keys[b, i * KT:(i + 1) * KT, :].T)
            vv = np.zeros((w * KT, 65), np.float32)
            vv[:vl, :D] = values[b, :vl, :]
            vv[:vl, D] = 1.0
            # [128 partitions, w, 65] flattened on the free axis
            blk[:, wqk:] = (
                vv.reshape(w, KT, 65).transpose(1, 0, 2).reshape(KT, w * 65))
            x0 += xw[j]
    return [{"data": data[c]} for c in range(NCORES)]


def kernel(queries, keys, values, valid_lens):
    global LAST_RESULTS
    queries = np.asarray(queries, dtype=np.float32)
    keys = np.asarray(keys, dtype=np.float32)
    values = np.asarray(values, dtype=np.float32)
    valid_lens = np.asarray(valid_lens)

    key = tuple(int(v) for v in valid_lens)
    if key not in _cache:
        nk, slots_nk, offs, assign = _schedule(valid_lens)
        nc = _build(slots_nk, offs)
        _cache[key] = (nc, slots_nk, offs, assign)
    nc, slots_nk, offs, assign = _cache[key]

    in_maps = _pack(queries, keys, values, valid_lens, slots_nk, offs, assign)
    res = run_bass_kernel_spmd(nc, in_maps, list(range(NCORES)))
    LAST_RESULTS = res

    out = np.empty((B, Q, D), np.float32)
    for c in range(NCORES):
        oc = res.results[c]["out"]          # [NSLOTS, D, QB]
        for j in range(NSLOTS):
            b, qb = assign[c][j]
            out[b, qb * QB:(qb + 1) * QB, :] = oc[j].T
    return out


# revision 65
# speedup vs baseline: 1.0044x; 1.0044x over previous
"""Masked dot-product attention (B=16, Q=K=2048, D=64) on 8 Trainium2 cores.

out = softmax(Q K^T / sqrt(64) + mask(valid_lens)) V, reproducing
reference.py's masked_softmax to ~7e-3 relative absmax (fp16 matmuls +
a bit-trick exp on two of the three exp engines, see below).

Sharding / load balance
-----------------------
Work units are (batch, 512-wide q-block): 64 units whose cost is
nk(b) = ceil(valid_len[b]/128) k-tiles. Units are sorted by nk descending and
dealt round-robin into 8 slots x 8 cores, so every core runs the *same*
static SPMD program (slot j processes NK_j = max-nk-of-its-rank-group
k-tiles) while the host packs each core's own data. Per-core inputs arrive
as packed [128, *] fp16 buffers per slot: [Q^T dup | K^T half-packed] and
[V_aug] (see _xw). Q|K and V ride separate DMAs, with V transfers trailing
so the early (ramp-limited) DMA bandwidth all feeds the S-matmul stream.

Device pipeline (inputs fp16; PSUM accumulates fp32)
----------------------------------------------------
  PE : S^T[128k, 512q] per k-tile = matmul(lhsT=K^T-tile, rhs=Q^T),
       contraction d=64, alternating k-tiles on PE row groups 0-63/64-127
  exp: P = exp(S^T/8) over the 3-bank PSUM group, split BY QUERY COLUMN
       across the two engines that can read PSUM (so each softmax row sees
       exactly one approximation and any per-engine scale cancels in the
       row's own denominator):
         - ACT cols [0:ACOL) of each 512 block: table-exact EXP ACTIVATE
         - DVE cols [ACOL:512): product bit-trick (below)
       (Pool/GPSIMD can't read PSUM and its tensor ops measured ~6-18x
       slower than modeled, so it only runs DMA issues here.)
  PE : O^T_aug[65, 512q] += matmul(lhsT=V_aug-tile[128,65], rhs=P-slice)

Product bit-trick exp (2 DVE-cycles/elem vs 1 ACT-cycle, but on otherwise
idle engines): exp(x/8) = 2^t (t = x*log2e/8) ~= f16bits(i) * f16bits(i-512)
where i = int16(x*64*log2e + BA). Each factor is a half-exponent Schraudolph
approximant 2^(t/2+c)*r(frac) with ripple r; the two factors sit exactly half
a mantissa-period apart, so the product's log-ripple cancels the odd
harmonics: +-1.5% max element error (vs +-6% for one trick). Empirically
7.3e-3 relative absmax end-to-end on the real inputs (tolerance 2e-2).
The int16 affine runs on fp32 PSUM directly; i-512 is an exact int
subtract; both factors multiply fp16->fp16 into the P tile.

O-matmuls lag one group behind S so the PE queue never head-of-line
blocks on exp. V_aug = [V | 1] with rows >= valid_len zeroed by the host
(exact masking, free denominator in row 64). A burst of dependency-free
tiny warm-up matmuls (reading the framework's pre-barrier const tile)
keeps the PE busy from t=0 until the first QK transfer lands, opening the
HAM clock gate to 2.4 GHz; the first COLD_GROUPS groups' O-matmuls defer
on top of that.

Close/division epilogue (all spread via a microtask FIFO)
---------------------------------------------------------
Per-unit close work is queued as thunks and pumped <=2-3 per group
boundary so no close ever inserts a multi-us bubble into an engine's exp
cadence: the [65,512] PSUM->SBUF numerator+denominator copy (DVE,
fp32->fp16) goes in 3 column chunks, the denominator-row hop rides the
Pool DMA queue. Reciprocals run on ACT as r = exp(-ln(d)) (Exp and Ln
share one pinned activation table set - no table switch): positions 0-5
as one batched [6,512] pair, broadcast across the 64 d-partitions via a
DRAM-bounce DMA (out-hop Sync / in-hop Pool overlap their SWDGE waits),
then fp16-multiplied and DMA'd out on alternating queues, all under the
stream. Position 6 runs the same chain solo with its two ACT ops in
separate groups and a ones-column PE broadcast. Only the final position
divides on the tail: Ln straight off its live PSUM denominator row, PE
broadcast, multiply, out. Output is fp16 [slot, 64, 512]; the host casts
and transposes O^T -> O while unsharding (numerators/denominators peak
~7.6e3 here, 8.6x under fp16 max).
"""

import sys

if "/opt/trn_rl_repo" not in sys.path:
    sys.path.insert(0, "/opt/trn_rl_repo")

import numpy as np

import bass_rust as _bass_rust
import concourse.bass as bass
import concourse.mybir as mybir
import concourse.tile as tile
from concourse import bacc
from concourse.bass_utils import run_bass_kernel_spmd
from concourse.hw_specs import get_activation_tables

B, Q, KLEN, D = 16, 2048, 2048, 64
QB = 512                      # q-block width per work unit
NCORES = 8
NSLOTS = (B * (Q // QB)) // NCORES   # 8 slots per core
KT = 128                      # k-tile height
GK = 3                        # k-tiles per exp group (3 PSUM banks)
NWARM = 40                    # dependency-free tiny PE warm-up matmuls
                              # (~28ns each, opening the HAM clock gate;
                              # more would risk queueing ahead of the
                              # first real S-matmul on the in-order PE)
COLD_GROUPS = 4               # exp groups whose O-matmuls defer to warm PE
O_LAG = 1                     # groups the O-matmuls trail the exp stream by
F32 = mybir.dt.float32
F16 = mybir.dt.float16
I16 = mybir.dt.int16
NPF16 = np.float16
AF = mybir.ActivationFunctionType
ALU = mybir.AluOpType

# --- exp column split (per 512-wide block) ---
# Pool/GPSIMD cannot read PSUM and its int16 tensor ops run ~18x slower
# than the DVE (software Q7 path, measured 14.8ns/elem), so the trick
# share lives entirely on the DVE: affine PSUM->int16, int subtract,
# fp16 multiply, ~3.2ns/elem total vs ACT's exact-table 0.83ns/elem.
ACOL = 404                    # ACT table-exact exp
DVCOL = QB - ACOL             # DVE product bit-trick
TRICK = DVCOL
LOG2E = 1.4426950408889634
TS_SCALE = 64.0 * LOG2E       # i = round(S*TS_SCALE + TS_BIAS)
TS_BIAS = 15817.1             # 15*1024 - Ca, Ca = -457.1 (centering: cosmetic)

LAST_RESULTS = None           # BassKernelResults of the most recent run

_cache: dict = {}


class _Bacc(bacc.Bacc):
    """Bacc whose activation-table fixpoint keeps every ACTIVATE on one
    set: Exp and Ln both live in natural_log_exp_and_others, but the
    default selector binds Exp to exp_and_others and then pays two ~1.3us
    ACT_TABLE_LOAD switches around the division's Ln. Restricting Exp/Ln
    to the combined set yields a single load."""

    def insert_act_table_loads(self):
        has_activation = any(
            isinstance(i, mybir.InstActivation)
            for b in self.main_func.blocks
            for i in b.instructions
        )
        if not has_activation:
            return
        tables = []
        for name, fns in get_activation_tables(self.m.arch).items():
            if name != "natural_log_exp_and_others":
                fns = fns - {AF.Exp, AF.Ln}
            tables.append((name, fns))
        _bass_rust.insert_act_table_loads(self, tables)


def _schedule(valid_lens):
    """Static work schedule from valid_lens (host-known at call time)."""
    nk = [max(1, -(-int(v) // KT)) for v in valid_lens]
    units = [(b, qb) for b in range(B) for qb in range(Q // QB)]
    units.sort(key=lambda u: (-nk[u[0]], u))
    slots_nk = [nk[units[NCORES * j][0]] for j in range(NSLOTS)]
    assign = [[units[NCORES * j + c] for j in range(NSLOTS)] for c in range(NCORES)]
    offs = np.concatenate([[0], np.cumsum(slots_nk)]).tolist()
    return nk, slots_nk, offs, assign


def _order(slots_nk):
    """Processing order: two smallest first (their little DMAs land fast,
    so the exp stream starts early), then the rest descending so the
    serial input-DMA stream always runs ahead of compute, and the biggest
    slot last so division batches hide under the final slot's stream.
    (Both a strictly ascending variant and an ascending-through-position-4
    variant measured worse - the ~2.5-3us of early-group stalls are the
    aggregate DMA ramp, not any one transfer's position, and moving big
    slots later just shifts the stall into the stream's second half.)"""
    asc = sorted(range(NSLOTS), key=lambda j: (slots_nk[j], j))
    return [asc[0], asc[1], asc[2], asc[6], asc[4], asc[3], asc[5], asc[7]]


def _xw(w):
    """Per-slot packed widths: [Q^T dup | K^T half-packed] + [V_aug].

    K^T tiles alternate between partition halves (even k-tile i in rows
    0-63 at column block i//2, odd in rows 64-127) - half the K DMA bytes
    of a full duplication while keeping the PE row-group pairing; only
    the small Q^T block is duplicated. (Shipping Q once + an on-chip dup
    DMA measured WORSE: the extra issue serialization and the dup
    dependency on odd k-tiles cost more than the 65KB/slot of wire.)"""
    kk = (w + 1) // 2
    return QB + kk * KT, w * 65


def _build(slots_nk, offs):
    """Build + compile the single SPMD program for the given slot profile."""
    order = _order(slots_nk)
    xw = [sum(_xw(w)) for w in slots_nk]
    xoffs = np.concatenate([[0], np.cumsum([xw[j] for j in order])]).tolist()
    rbatch = [0, 1, 2, 3, 4, 5]   # one division batch: Ln/Exp cost is per
                                  # batch (free-dim cycles), not per unit

    nc = _Bacc()
    data_d = nc.dram_tensor("data", [2 * D, xoffs[-1]], F16,
                            kind="ExternalInput").ap()
    # fp16 output: the host casts to fp32 while unsharding; numerators and
    # denominators peak ~7.6e3 on this distribution, 8.6x under fp16 max
    out_d = nc.dram_tensor("out", [NSLOTS, D, QB], F16, kind="ExternalOutput").ap()

    with tile.TileContext(nc) as tc:
        with (
            tc.tile_pool(name="spool", bufs=8) as spool,
            tc.tile_pool(name="vpool", bufs=8) as vpool,
            tc.tile_pool(name="ppool", bufs=8) as ppool,
            tc.tile_pool(name="tpool", bufs=3) as tpool,
            tc.tile_pool(name="epool", bufs=3) as epool,
            tc.tile_pool(name="gpool", bufs=1) as gpool,
            tc.tile_pool(name="opool", bufs=8) as opool,
            tc.tile_pool(name="dpool", bufs=2, space="DRAM") as dpool,
            tc.tile_pool(name="psum_s", bufs=2, space="PSUM") as psum_s,
            tc.tile_pool(name="psum_o", bufs=2, space="PSUM") as psum_o,
        ):
            dn_tile = gpool.tile([len(rbatch), QB], F16, name="dn", tag="dn")
            ones_sb = gpool.tile([1, D], F16, name="ones", tag="ones")
            nc.vector.memset(ones_sb, 1.0)
            # tiny first DMA warms the Sync HWDGE queue (its ~1.3us
            # activation latency) before the first real input transfer
            wdma = gpool.tile([1, D], F16, name="wdma", tag="wdma")
            nc.sync.dma_start(out=wdma, in_=data_d[0:1, 0:D])

            # PE warm-up: dependency-free matmuls from t=0 keep the PE busy
            # through the HAM activity window so the clock-gate opens to
            # 2.4 GHz before (or soon after) the first real matmul. They
            # read the framework's const tile (memset BEFORE the preamble
            # barrier) so the first one issues the moment the PE queue
            # reaches our block - no wait on our own memsets.
            cbf = nc.const_aps.aps[(mybir.dt.bfloat16, 1.0)]
            warm_ps = psum_o.tile([65, QB], F32, name="warm_ps", tag="po")
            for _ in range(NWARM):
                nc.tensor.matmul(warm_ps[0:1, 0:1], lhsT=cbf, rhs=cbf,
                                 start=True, stop=True)

            o_tiles = {}
            # Per-unit close work (PSUM->SBUF copy chunks, denominator-row
            # hops, the division batch, final multiplies) is spread across
            # group boundaries via this FIFO of thunks: at most MICRO_PUMP
            # of them run per boundary, so no single close ever inserts a
            # multi-us bubble into the DVE/ACT exp cadence.
            microq = []      # (min_gi, fn): fn emits once cur group >= min_gi
            cur = {"gi": 0}
            MICRO_PUMP = 2

            def pump_micro(n=MICRO_PUMP, drain=False):
                ran = 0
                i = 0
                while i < len(microq) and (ran < n or drain):
                    if drain or microq[i][0] <= cur["gi"]:
                        microq.pop(i)[1]()
                        ran += 1
                    else:
                        i += 1

            rb_holder = {}

            def div_stage1():
                # r = exp(-ln(d)) on ACT, one batched [6,512] op pair
                # (free-dim cycles are paid once per batch); both functions
                # share the pinned activation table set, so no table
                # switch. Broadcast across the 64 d-partitions via a DRAM
                # bounce (DRAM is flat, so the read-back replicates with a
                # stride-0 leading dim). Both hops ride the Pool queue so
                # they never block Sync input issues.
                n = len(rbatch)
                lnd = epool.tile([n, QB], F32, tag="lnd")
                nc.scalar.activation(lnd, dn_tile, AF.Ln)
                r_sb = epool.tile([n, QB], F16, tag="r")
                nc.scalar.activation(r_sb, lnd, AF.Exp, scale=-1.0)
                scratch = dpool.tile([n, QB], F16, tag="scr")
                # out-hop on Sync (idle mid-stream): overlapping the two
                # hops across queues shaves the serialized SWDGE waits
                nc.sync.dma_start(out=scratch, in_=r_sb)
                rb_sb = epool.tile([D, n, QB], F16, tag="rb")
                bcast_src = bass.AP(
                    tensor=scratch.tensor,
                    offset=scratch.offset,
                    ap=[[0, D]] + [list(a) for a in scratch.ap],
                )
                nc.gpsimd.dma_start(out=rb_sb, in_=bcast_src)
                rb_holder["rb"] = rb_sb

            def stage2_one(ui, jj):
                def f():
                    oo_sb = opool.tile([D, QB], F16, tag="oo")
                    nc.vector.tensor_mul(oo_sb, o_tiles[jj],
                                         rb_holder["rb"][:, ui, :])
                    # alternate queues: 8 back-to-back issues on one queue
                    # serialize ~5us of tail otherwise
                    eng = nc.sync if jj % 2 == 0 else nc.gpsimd
                    eng.dma_start(out=out_d[order[jj]], in_=oo_sb)
                return f

            slot_ctx = {}

            def open_qk(jidx):
                j = order[jidx]
                w = slots_nk[j]
                wqk, wv = _xw(w)
                qk_sb = spool.tile([2 * D, wqk], F16, tag="xqk")
                # early QK issues alternate Sync / ACT-HWDGE: the ACT queue
                # is idle until its first exp (~10.7us), so pairing the
                # issues halves the ~0.6us-each serialization right where
                # the stream is input-supply-limited
                eng = nc.scalar if jidx in (1, 3) else nc.sync
                eng.dma_start(
                    out=qk_sb,
                    in_=data_d[:, xoffs[jidx]:xoffs[jidx] + wqk])
                po = psum_o.tile([65, QB], F32, tag="po")
                slot_ctx[jidx] = [qk_sb, None, po, w]

            def open_v(jidx):
                j = order[jidx]
                w = slots_nk[j]
                wqk, wv = _xw(w)
                xv_sb = vpool.tile([2 * D, wv], F16, tag="xv")
                nc.sync.dma_start(
                    out=xv_sb,
                    in_=data_d[:, xoffs[jidx] + wqk:xoffs[jidx] + wqk + wv])
                slot_ctx[jidx][1] = xv_sb

            def close_slot(jidx, last=False):
                _, _, po, _ = slot_ctx[jidx]
                if jidx == NSLOTS - 1:
                    # final position: the whole division chain runs inline
                    # at the close - Ln straight off the PSUM denominator
                    # row in parallel with the DVE numerator copy, then
                    # r = exp(-ln(d)), ones-column PE broadcast, multiply,
                    # out. All tiles are dedicated (gpool) so no pool-
                    # rotation dependency can delay this tail.
                    lnd = gpool.tile([1, QB], F32, name=f"lnd{jidx}",
                                     tag=f"lnd{jidx}")
                    nc.scalar.activation(lnd, po[64:65, :], AF.Ln)
                    oa_sb = gpool.tile([D, QB], F16, name=f"oa{jidx}",
                                       tag=f"oa{jidx}")
                    nc.vector.tensor_copy(oa_sb, po[0:64, :])
                    r16 = gpool.tile([1, QB], F16, name=f"r16{jidx}",
                                     tag=f"r16{jidx}")
                    nc.scalar.activation(r16, lnd, AF.Exp, scale=-1.0)
                    # broadcast target from the psum_o pool: the rotation
                    # lands it on this unit's own accumulator bank, whose
                    # readers are this chain's own upstream
                    bc = psum_o.tile([65, QB], F32, tag="po")
                    nc.tensor.matmul(bc[0:D, :], lhsT=ones_sb,
                                     rhs=r16, start=True, stop=True)
                    oo_sb = opool.tile([D, QB], F16, tag="oo")
                    nc.vector.tensor_mul(oo_sb, oa_sb, bc[0:D, :])
                    nc.sync.dma_start(out=out_d[order[jidx]], in_=oo_sb)
                    return
                # the [65,512] PSUM->SBUF copy (fp32->fp16 in the op, DVE:
                # ACT is the exp-cadence-critical engine) moves numerator +
                # denominator together and frees the PSUM bank. For units
                # followed by a long slot it's split into column chunks and
                # pumped one per group boundary so the DVE exp chain never
                # takes a ~700ns hit in one group; the first two (short)
                # positions copy whole - their PSUM bank is re-needed a
                # single group later.
                oa_sb = gpool.tile([65, QB], F16, name=f"oa{jidx}",
                                   tag=f"oa{jidx}")
                # the batch-closing position copies whole so its
                # denominator row is ready before the batch Ln needs it
                nchunks = 1 if (jidx < 2 or jidx == rbatch[-1]) else 3
                step = -(-QB // nchunks)
                for c0 in range(0, QB, step):
                    c1 = min(QB, c0 + step)
                    if nchunks == 1:
                        nc.vector.tensor_copy(oa_sb[:, c0:c1], po[:, c0:c1])
                    else:
                        microq.append(
                            (0, lambda a=c0, b=c1:
                             nc.vector.tensor_copy(oa_sb[:, a:b],
                                                   po[:, a:b])))
                o_tiles[jidx] = oa_sb[0:D, :]
                if jidx in rbatch:
                    ri = rbatch.index(jidx)
                    if jidx == rbatch[-1]:
                        # immediate hop + near-term division: the whole
                        # chain (Ln/Exp/bounce/mults/out-DMAs) has ~5us of
                        # queue+DMA latency and must finish under the
                        # stream, not after it
                        nc.gpsimd.dma_start(out=dn_tile[ri:ri + 1, :],
                                            in_=oa_sb[64:65, :])
                        microq.append((cur["gi"] + 2, div_stage1))
                        for ui, jj in enumerate(rbatch):
                            microq.append((cur["gi"] + 4 + ui // 2,
                                           stage2_one(ui, jj)))
                    else:
                        microq.append(
                            (0, lambda: nc.gpsimd.dma_start(
                                out=dn_tile[ri:ri + 1, :],
                                in_=oa_sb[64:65, :])))
                else:
                    # second-to-last position: solo division via microq -
                    # its two ACT ops land in separate groups (each mostly
                    # absorbed by per-group ACT slack) instead of a 1.3us
                    # block that stalls the exp cadence; the broadcast is a
                    # single PE ones-matmul once r16 is ready.
                    g0 = cur["gi"]
                    lnd6 = gpool.tile([1, QB], F32, name="lnd6", tag="lnd6")
                    r166 = gpool.tile([1, QB], F16, name="r166", tag="r166")
                    microq.append(
                        (g0 + 2, lambda: nc.scalar.activation(
                            lnd6, oa_sb[64:65, :], AF.Ln)))
                    microq.append(
                        (g0 + 3, lambda: nc.scalar.activation(
                            r166, lnd6, AF.Exp, scale=-1.0)))

                    def fin6():
                        bc = psum_o.tile([65, QB], F32, tag="po")
                        nc.tensor.matmul(bc[0:D, :], lhsT=ones_sb,
                                         rhs=r166, start=True, stop=True)
                        oo_sb = opool.tile([D, QB], F16, tag="oo")
                        nc.vector.tensor_mul(oo_sb, o_tiles[jidx],
                                             bc[0:D, :])
                        nc.gpsimd.dma_start(out=out_d[order[jidx]],
                                            in_=oo_sb)
                    microq.append((g0 + 4, fin6))

            # all input DMAs are issued up front (spool/vpool hold one
            # buffer per slot, so no rotation waits). Q|K transfers lead
            # and V transfers trail two slots behind.
            open_qk(0)
            open_qk(1)
            open_qk(2)
            for jidx in range(3, NSLOTS):
                open_v(jidx - 3)
                open_qk(jidx)
            for jidx in range(NSLOTS - 3, NSLOTS):
                open_v(jidx)

            # flat k-tile schedule: exp groups are GK consecutive k-tiles
            # REGARDLESS of slot boundaries, so every exp op but the last
            # runs at the full N=1536 and slot transitions produce no
            # short-group hiccups
            flat = []
            for jidx, j in enumerate(order):
                w = slots_nk[j]
                for ki in range(w):
                    flat.append((jidx, ki, ki == w - 1))
            # group 0 covers slot 0 alone so the first exp gates only on
            # the first (smallest) QK transfer
            w0 = min(slots_nk[order[0]], GK)
            fgroups = [flat[:w0]] + [flat[i:i + GK]
                                     for i in range(w0, len(flat), GK)]

            def run_group(items, last=False):
                for pj, ki, closes, ph, p_prev in items:
                    _, pxv, ppo, pw = slot_ctx[pj]
                    pva = pxv.rearrange("p (w c) -> p w c", c=65)
                    nc.tensor.matmul(
                        ppo,
                        lhsT=pva[:, ki, :],
                        rhs=p_prev[:, ph * QB:(ph + 1) * QB],
                        start=(ki == 0), stop=(ki == pw - 1),
                    )
                    if closes:
                        close_slot(pj, last=last)

            def emit_exp(ps, p_sb, g, force_act=False):
                """Two-engine exp over a [128, g*512] PSUM group, split by
                query column (see module docstring). force_act runs the
                whole group on ACT - used for the first and last groups so
                the pipeline's ends don't wait on the DVE chain."""
                ww = g * QB
                if force_act:
                    nc.scalar.activation(p_sb[:, :ww], ps[:, :ww],
                                         AF.Exp, scale=0.125)
                    return
                ps3 = ps[:, :ww].rearrange("p (g q) -> p g q", q=QB)
                p3 = p_sb[:, :ww].rearrange("p (g q) -> p g q", q=QB)
                # ACT: table-exact exp on its column share
                nc.scalar.activation(
                    p3[:, :, 0:ACOL], ps3[:, :, 0:ACOL], AF.Exp, scale=0.125)
                # DVE: product bit-trick on its column share
                ia = tpool.tile([128, GK * TRICK], I16, tag="ia")
                ib = tpool.tile([128, GK * TRICK], I16, tag="ib")
                ia3 = ia[:, :g * TRICK].rearrange("p (g q) -> p g q", q=TRICK)
                ib3 = ib[:, :g * TRICK].rearrange("p (g q) -> p g q", q=TRICK)
                iaf = ia[:, :g * TRICK].bitcast(F16).rearrange(
                    "p (g q) -> p g q", q=TRICK)
                ibf = ib[:, :g * TRICK].bitcast(F16).rearrange(
                    "p (g q) -> p g q", q=TRICK)
                nc.vector.tensor_scalar(
                    ia3, ps3[:, :, ACOL:],
                    TS_SCALE, TS_BIAS, ALU.mult, ALU.add)
                nc.vector.tensor_scalar(ib3, ia3, 512.0, None, ALU.subtract)
                nc.vector.tensor_mul(p3[:, :, ACOL:], iaf, ibf)

            # While the PE HAM clock-gate is still cold (1.2 GHz), S
            # matmuls alone just keep pace with the exp stream, but S+O
            # would stall it: O-groups from the first COLD_GROUPS groups
            # defer entirely, then the backlog drains two-per-group down
            # to a steady O_LAG-group lag.
            ngroups = len(fgroups)
            oqueue = []   # deferred O-group batches, oldest first
            for gi, grp in enumerate(fgroups):
                cur["gi"] = gi
                g = len(grp)
                ps = psum_s.tile([128, GK * QB], F32, tag="ps")
                for i, (jidx, ki, _) in enumerate(grp):
                    qk_sb = slot_ctx[jidx][0]
                    qt_sb = qk_sb[:, 0:QB]
                    kt_sb = qk_sb[:, QB:]
                    rg = (ki % 2) * D   # row-group half = k-tile parity
                    nc.tensor.matmul(
                        ps[:, i * QB:(i + 1) * QB],
                        lhsT=kt_sb[rg:rg + D, (ki // 2) * KT:
                                   (ki // 2 + 1) * KT],
                        rhs=qt_sb[rg:rg + D, :],
                        start=True, stop=True,
                        tile_position=(rg, 0),
                    )
                if gi > COLD_GROUPS:
                    drains = (2 if len(oqueue) > O_LAG + 1 else
                              1 if len(oqueue) > O_LAG else 0)
                    for _ in range(drains):
                        run_group(oqueue.pop(0))
                p_sb = ppool.tile([128, GK * QB], F16, tag="p")
                emit_exp(ps, p_sb, g, force_act=(gi == 0))
                oqueue.append([(jidx, ki, closes, i, p_sb)
                               for i, (jidx, ki, closes) in enumerate(grp)])
                # pump harder near the end so no close work spills past
                # the last O-matmuls into the tail
                pump_micro(3 if gi >= ngroups - 6 else MICRO_PUMP)
            cur["gi"] = ngroups + 10
            pump_micro(drain=True)
            while oqueue:
                run_group(oqueue.pop(0), last=(len(oqueue) == 1))

    nc.compile()
    return nc


def _pack(queries, keys, values, valid_lens, slots_nk, offs, assign):
    order = _order(slots_nk)
    xw = [sum(_xw(w)) for w in slots_nk]
    tot = sum(xw)
    data = np.zeros((NCORES, 2 * D, tot), NPF16)
    for c in range(NCORES):
        x0 = 0
        for p, j in enumerate(order):
            b, qb = assign[c][j]
            w = slots_nk[j]
            wqk, wv = _xw(w)
            vl = int(valid_lens[b])
            blk = data[c, :, x0:x0 + xw[j]]
            qt = queries[b, qb * QB:(qb + 1) * QB, :].T      # [D, QB]
            blk[:D, 0:QB] = qt
            blk[D:, 0:QB] = qt
            # K^T tiles alternate partition halves: even k-tile i in rows
            # 0-63, odd in rows 64-127, both at column block i//2
            for i in range(w):
                half = (i % 2) * D
                c0 = QB + (i // 2) * KT
                blk[half:half + D, c0:c0 + KT] = (
                    keys[b, i * KT:(i + 1) * KT, :].T)
            vv = np.zeros((w * KT, 65), np.float32)
            vv[:vl, :D] = values[b, :vl, :]
            vv[:vl, D] = 1.0
            # [128 partitions, w, 65] flattened on the free axis
            blk[:, wqk:] = (
                vv.reshape(w, KT, 65).transpose(1, 0, 2).reshape(KT, w * 65))
            x0 += xw[j]
    return [{"data": data[c]} for c in range(NCORES)]


def kernel(queries, keys, values, valid_lens):
    global LAST_RESULTS
    queries = np.asarray(queries, dtype=np.float32)
    keys = np.asarray(keys, dtype=np.float32)
    values = np.asarray(values, dtype=np.float32)
    valid_lens = np.asarray(valid_lens)

    key = tuple(int(v) for v in valid_lens)
    if key not in _cache:
        nk, slots_nk, offs, assign = _schedule(valid_lens)
        nc = _build(slots_nk, offs)
        _cache[key] = (nc, slots_nk, offs, assign)
    nc, slots_nk, offs, assign = _cache[key]

    in_maps = _pack(queries, keys, values, valid_lens, slots_nk, offs, assign)
    res = run_bass_kernel_spmd(nc, in_maps, list(range(NCORES)))
    LAST_RESULTS = res

    out = np.empty((B, Q, D), np.float32)
    for c in range(NCORES):
        oc = res.results[c]["out"]          # [NSLOTS, D, QB]
        for j in range(NSLOTS):
            b, qb = assign[c][j]
            out[b, qb * QB:(qb + 1) * QB, :] = oc[j].T
    return out


# revision 67
# speedup vs baseline: 1.0113x; 1.0068x over previous
"""Masked dot-product attention (B=16, Q=K=2048, D=64) on 8 Trainium2 cores.

out = softmax(Q K^T / sqrt(64) + mask(valid_lens)) V, reproducing
reference.py's masked_softmax to ~7e-3 relative absmax (fp16 matmuls +
a bit-trick exp on two of the three exp engines, see below).

Sharding / load balance
-----------------------
Work units are (batch, 512-wide q-block): 64 units whose cost is
nk(b) = ceil(valid_len[b]/128) k-tiles. Units are sorted by nk descending and
dealt round-robin into 8 slots x 8 cores, so every core runs the *same*
static SPMD program (slot j processes NK_j = max-nk-of-its-rank-group
k-tiles) while the host packs each core's own data. Per-core inputs arrive
as packed [128, *] fp16 buffers per slot: [Q^T dup | K^T half-packed] and
[V_aug] (see _xw). Q|K and V ride separate DMAs, with V transfers trailing
so the early (ramp-limited) DMA bandwidth all feeds the S-matmul stream.

Device pipeline (inputs fp16; PSUM accumulates fp32)
----------------------------------------------------
  PE : S^T[128k, 512q] per k-tile = matmul(lhsT=K^T-tile, rhs=Q^T),
       contraction d=64, alternating k-tiles on PE row groups 0-63/64-127
  exp: P = exp(S^T/8) over the 3-bank PSUM group, split BY QUERY COLUMN
       across the two engines that can read PSUM (so each softmax row sees
       exactly one approximation and any per-engine scale cancels in the
       row's own denominator):
         - ACT cols [0:ACOL) of each 512 block: table-exact EXP ACTIVATE
         - DVE cols [ACOL:512): product bit-trick (below)
       (Pool/GPSIMD can't read PSUM and its tensor ops measured ~6-18x
       slower than modeled, so it only runs DMA issues here.)
  PE : O^T_aug[65, 512q] += matmul(lhsT=V_aug-tile[128,65], rhs=P-slice)

Product bit-trick exp (2 DVE-cycles/elem vs 1 ACT-cycle, but on otherwise
idle engines): exp(x/8) = 2^t (t = x*log2e/8) ~= f16bits(i) * f16bits(i-512)
where i = int16(x*64*log2e + BA). Each factor is a half-exponent Schraudolph
approximant 2^(t/2+c)*r(frac) with ripple r; the two factors sit exactly half
a mantissa-period apart, so the product's log-ripple cancels the odd
harmonics: +-1.5% max element error (vs +-6% for one trick). Empirically
7.3e-3 relative absmax end-to-end on the real inputs (tolerance 2e-2).
The int16 affine runs on fp32 PSUM directly; i-512 is an exact int
subtract; both factors multiply fp16->fp16 into the P tile.

O-matmuls lag one group behind S so the PE queue never head-of-line
blocks on exp. V_aug = [V | 1] with rows >= valid_len zeroed by the host
(exact masking, free denominator in row 64). A burst of dependency-free
tiny warm-up matmuls (reading the framework's pre-barrier const tile)
keeps the PE busy from t=0 until the first QK transfer lands, opening the
HAM clock gate to 2.4 GHz; the first COLD_GROUPS groups' O-matmuls defer
on top of that.

Close/division epilogue (all spread via a microtask FIFO)
---------------------------------------------------------
Per-unit close work is queued as thunks and pumped <=2-3 per group
boundary so no close ever inserts a multi-us bubble into an engine's exp
cadence: the [65,512] PSUM->SBUF numerator+denominator copy (DVE,
fp32->fp16) goes in 3 column chunks, the denominator-row hop rides the
Pool DMA queue. Reciprocals run on ACT as r = exp(-ln(d)) (Exp and Ln
share one pinned activation table set - no table switch): positions 0-5
as one batched [6,512] pair, broadcast across the 64 d-partitions via a
DRAM-bounce DMA (out-hop Sync / in-hop Pool overlap their SWDGE waits),
then fp16-multiplied and DMA'd out on alternating queues, all under the
stream. Position 6 runs the same chain solo with its two ACT ops in
separate groups and a ones-column PE broadcast. Only the final position
divides on the tail: Ln straight off its live PSUM denominator row, PE
broadcast, multiply, out. Output is fp16 [slot, 64, 512]; the host casts
and transposes O^T -> O while unsharding (numerators/denominators peak
~7.6e3 here, 8.6x under fp16 max).
"""

import sys

if "/opt/trn_rl_repo" not in sys.path:
    sys.path.insert(0, "/opt/trn_rl_repo")

import numpy as np

import bass_rust as _bass_rust
import concourse.bass as bass
import concourse.mybir as mybir
import concourse.tile as tile
from concourse import bacc
from concourse.bass_utils import run_bass_kernel_spmd

# NOTE: compiling with walrus --max-sem-num=176 (to shrink the ~6.5us
# fixed postamble that serially zeroes all 253 semaphores) passes the
# verifier but fails at runtime - the capped space collides with
# NRT-reserved high semaphores. Do not retry without a runtime fix.
from concourse.hw_specs import get_activation_tables

B, Q, KLEN, D = 16, 2048, 2048, 64
QB = 512                      # q-block width per work unit
NCORES = 8
NSLOTS = (B * (Q // QB)) // NCORES   # 8 slots per core
KT = 128                      # k-tile height
GK = 3                        # k-tiles per exp group (3 PSUM banks)
NWARM = 40                    # dependency-free tiny PE warm-up matmuls
                              # (~28ns each, opening the HAM clock gate;
                              # more would risk queueing ahead of the
                              # first real S-matmul on the in-order PE)
COLD_GROUPS = 4               # exp groups whose O-matmuls defer to warm PE
O_LAG = 1                     # groups the O-matmuls trail the exp stream by
F32 = mybir.dt.float32
F16 = mybir.dt.float16
I16 = mybir.dt.int16
NPF16 = np.float16
AF = mybir.ActivationFunctionType
ALU = mybir.AluOpType

# --- exp column split (per 512-wide block) ---
# Pool/GPSIMD cannot read PSUM and its int16 tensor ops run ~18x slower
# than the DVE (software Q7 path, measured 14.8ns/elem), so the trick
# share lives entirely on the DVE: affine PSUM->int16, int subtract,
# fp16 multiply, ~3.2ns/elem total vs ACT's exact-table 0.83ns/elem.
ACOL = 404                    # ACT table-exact exp
DVCOL = QB - ACOL             # DVE product bit-trick
TRICK = DVCOL
LOG2E = 1.4426950408889634
TS_SCALE = 64.0 * LOG2E       # i = round(S*TS_SCALE + TS_BIAS)
TS_BIAS = 15817.1             # 15*1024 - Ca, Ca = -457.1 (centering: cosmetic)

LAST_RESULTS = None           # BassKernelResults of the most recent run

_cache: dict = {}


class _Bacc(bacc.Bacc):
    """Bacc whose activation-table fixpoint keeps every ACTIVATE on one
    set: Exp and Ln both live in natural_log_exp_and_others, but the
    default selector binds Exp to exp_and_others and then pays two ~1.3us
    ACT_TABLE_LOAD switches around the division's Ln. Restricting Exp/Ln
    to the combined set yields a single load."""

    def insert_act_table_loads(self):
        has_activation = any(
            isinstance(i, mybir.InstActivation)
            for b in self.main_func.blocks
            for i in b.instructions
        )
        if not has_activation:
            return
        tables = []
        for name, fns in get_activation_tables(self.m.arch).items():
            if name != "natural_log_exp_and_others":
                fns = fns - {AF.Exp, AF.Ln}
            tables.append((name, fns))
        _bass_rust.insert_act_table_loads(self, tables)


def _schedule(valid_lens):
    """Static work schedule from valid_lens (host-known at call time)."""
    nk = [max(1, -(-int(v) // KT)) for v in valid_lens]
    units = [(b, qb) for b in range(B) for qb in range(Q // QB)]
    units.sort(key=lambda u: (-nk[u[0]], u))
    slots_nk = [nk[units[NCORES * j][0]] for j in range(NSLOTS)]
    assign = [[units[NCORES * j + c] for j in range(NSLOTS)] for c in range(NCORES)]
    offs = np.concatenate([[0], np.cumsum(slots_nk)]).tolist()
    return nk, slots_nk, offs, assign


def _order(slots_nk):
    """Processing order: two smallest first (their little DMAs land fast,
    so the exp stream starts early), then the rest descending so the
    serial input-DMA stream always runs ahead of compute, and the biggest
    slot last so division batches hide under the final slot's stream.
    (Both a strictly ascending variant and an ascending-through-position-4
    variant measured worse - the ~2.5-3us of early-group stalls are the
    aggregate DMA ramp, not any one transfer's position, and moving big
    slots later just shifts the stall into the stream's second half.)"""
    asc = sorted(range(NSLOTS), key=lambda j: (slots_nk[j], j))
    return [asc[0], asc[1], asc[2], asc[6], asc[4], asc[3], asc[5], asc[7]]


def _xw(w):
    """Per-slot packed widths: [Q^T dup | K^T half-packed] + [V_aug].

    K^T tiles alternate between partition halves (even k-tile i in rows
    0-63 at column block i//2, odd in rows 64-127) - half the K DMA bytes
    of a full duplication while keeping the PE row-group pairing; only
    the small Q^T block is duplicated. (Shipping Q once + an on-chip dup
    DMA measured WORSE: the extra issue serialization and the dup
    dependency on odd k-tiles cost more than the 65KB/slot of wire.)"""
    kk = (w + 1) // 2
    return QB + kk * KT, w * 65


def _build(slots_nk, offs):
    """Build + compile the single SPMD program for the given slot profile."""
    order = _order(slots_nk)
    xw = [sum(_xw(w)) for w in slots_nk]
    xoffs = np.concatenate([[0], np.cumsum([xw[j] for j in order])]).tolist()
    rbatch = [0, 1, 2, 3, 4, 5]   # one division batch: Ln/Exp cost is per
                                  # batch (free-dim cycles), not per unit

    nc = _Bacc()
    data_d = nc.dram_tensor("data", [2 * D, xoffs[-1]], F16,
                            kind="ExternalInput").ap()
    # fp16 output: the host casts to fp32 while unsharding; numerators and
    # denominators peak ~7.6e3 on this distribution, 8.6x under fp16 max
    out_d = nc.dram_tensor("out", [NSLOTS, D, QB], F16, kind="ExternalOutput").ap()

    with tile.TileContext(nc) as tc:
        with (
            tc.tile_pool(name="spool", bufs=8) as spool,
            tc.tile_pool(name="vpool", bufs=8) as vpool,
            tc.tile_pool(name="ppool", bufs=8) as ppool,
            tc.tile_pool(name="tpool", bufs=3) as tpool,
            tc.tile_pool(name="epool", bufs=3) as epool,
            tc.tile_pool(name="gpool", bufs=1) as gpool,
            tc.tile_pool(name="opool", bufs=8) as opool,
            tc.tile_pool(name="dpool", bufs=2, space="DRAM") as dpool,
            tc.tile_pool(name="psum_s", bufs=2, space="PSUM") as psum_s,
            tc.tile_pool(name="psum_o", bufs=2, space="PSUM") as psum_o,
        ):
            dn_tile = gpool.tile([len(rbatch), QB], F16, name="dn", tag="dn")
            ones_sb = gpool.tile([1, D], F16, name="ones", tag="ones")
            nc.vector.memset(ones_sb, 1.0)
            # tiny first DMA warms the Sync HWDGE queue (its ~1.3us
            # activation latency) before the first real input transfer
            wdma = gpool.tile([1, D], F16, name="wdma", tag="wdma")
            nc.sync.dma_start(out=wdma, in_=data_d[0:1, 0:D])

            # PE warm-up: dependency-free matmuls from t=0 keep the PE busy
            # through the HAM activity window so the clock-gate opens to
            # 2.4 GHz before (or soon after) the first real matmul. They
            # read the framework's const tile (memset BEFORE the preamble
            # barrier) so the first one issues the moment the PE queue
            # reaches our block - no wait on our own memsets.
            cbf = nc.const_aps.aps[(mybir.dt.bfloat16, 1.0)]
            warm_ps = psum_o.tile([65, QB], F32, name="warm_ps", tag="po")
            for _ in range(NWARM):
                nc.tensor.matmul(warm_ps[0:1, 0:1], lhsT=cbf, rhs=cbf,
                                 start=True, stop=True)

            o_tiles = {}
            # Per-unit close work (PSUM->SBUF copy chunks, denominator-row
            # hops, the division batch, final multiplies) is spread across
            # group boundaries via this FIFO of thunks: at most MICRO_PUMP
            # of them run per boundary, so no single close ever inserts a
            # multi-us bubble into the DVE/ACT exp cadence.
            microq = []      # (min_gi, fn): fn emits once cur group >= min_gi
            cur = {"gi": 0}
            MICRO_PUMP = 2

            def pump_micro(n=MICRO_PUMP, drain=False):
                ran = 0
                i = 0
                while i < len(microq) and (ran < n or drain):
                    if drain or microq[i][0] <= cur["gi"]:
                        microq.pop(i)[1]()
                        ran += 1
                    else:
                        i += 1

            rb_holder = {}

            def div_stage1():
                # r = exp(-ln(d)) on ACT, one batched [6,512] op pair
                # (free-dim cycles are paid once per batch); both functions
                # share the pinned activation table set, so no table
                # switch. Broadcast across the 64 d-partitions via a DRAM
                # bounce (DRAM is flat, so the read-back replicates with a
                # stride-0 leading dim). Both hops ride the Pool queue so
                # they never block Sync input issues.
                n = len(rbatch)
                lnd = epool.tile([n, QB], F32, tag="lnd")
                nc.scalar.activation(lnd, dn_tile, AF.Ln)
                r_sb = epool.tile([n, QB], F16, tag="r")
                nc.scalar.activation(r_sb, lnd, AF.Exp, scale=-1.0)
                scratch = dpool.tile([n, QB], F16, tag="scr")
                # out-hop on Sync (idle mid-stream): overlapping the two
                # hops across queues shaves the serialized SWDGE waits
                nc.sync.dma_start(out=scratch, in_=r_sb)
                rb_sb = epool.tile([D, n, QB], F16, tag="rb")
                bcast_src = bass.AP(
                    tensor=scratch.tensor,
                    offset=scratch.offset,
                    ap=[[0, D]] + [list(a) for a in scratch.ap],
                )
                nc.gpsimd.dma_start(out=rb_sb, in_=bcast_src)
                rb_holder["rb"] = rb_sb

            def stage2_one(ui, jj):
                def f():
                    oo_sb = opool.tile([D, QB], F16, tag="oo")
                    nc.vector.tensor_mul(oo_sb, o_tiles[jj],
                                         rb_holder["rb"][:, ui, :])
                    # alternate queues: 8 back-to-back issues on one queue
                    # serialize ~5us of tail otherwise
                    eng = nc.sync if jj % 2 == 0 else nc.gpsimd
                    eng.dma_start(out=out_d[order[jj]], in_=oo_sb)
                return f

            slot_ctx = {}

            def open_qk(jidx):
                j = order[jidx]
                w = slots_nk[j]
                wqk, wv = _xw(w)
                qk_sb = spool.tile([2 * D, wqk], F16, tag="xqk")
                # early QK issues alternate Sync / ACT-HWDGE: the ACT queue
                # is idle until its first exp (~10.7us), so pairing the
                # issues halves the ~0.6us-each serialization right where
                # the stream is input-supply-limited
                eng = nc.scalar if jidx in (1, 3) else nc.sync
                eng.dma_start(
                    out=qk_sb,
                    in_=data_d[:, xoffs[jidx]:xoffs[jidx] + wqk])
                po = psum_o.tile([65, QB], F32, tag="po")
                slot_ctx[jidx] = [qk_sb, None, po, w]

            def open_v(jidx):
                j = order[jidx]
                w = slots_nk[j]
                wqk, wv = _xw(w)
                xv_sb = vpool.tile([2 * D, wv], F16, tag="xv")
                nc.sync.dma_start(
                    out=xv_sb,
                    in_=data_d[:, xoffs[jidx] + wqk:xoffs[jidx] + wqk + wv])
                slot_ctx[jidx][1] = xv_sb

            def close_slot(jidx, last=False):
                _, _, po, _ = slot_ctx[jidx]
                if jidx == NSLOTS - 1:
                    # final position: the whole division chain runs inline
                    # at the close - Ln straight off the PSUM denominator
                    # row in parallel with the DVE numerator copy, then
                    # r = exp(-ln(d)), ones-column PE broadcast, multiply,
                    # out. All tiles are dedicated (gpool) so no pool-
                    # rotation dependency can delay this tail.
                    lnd = gpool.tile([1, QB], F32, name=f"lnd{jidx}",
                                     tag=f"lnd{jidx}")
                    nc.scalar.activation(lnd, po[64:65, :], AF.Ln)
                    oa_sb = gpool.tile([D, QB], F16, name=f"oa{jidx}",
                                       tag=f"oa{jidx}")
                    nc.vector.tensor_copy(oa_sb, po[0:64, :])
                    r16 = gpool.tile([1, QB], F16, name=f"r16{jidx}",
                                     tag=f"r16{jidx}")
                    nc.scalar.activation(r16, lnd, AF.Exp, scale=-1.0)
                    # broadcast target from the psum_o pool: the rotation
                    # lands it on this unit's own accumulator bank, whose
                    # readers are this chain's own upstream
                    bc = psum_o.tile([65, QB], F32, tag="po")
                    nc.tensor.matmul(bc[0:D, :], lhsT=ones_sb,
                                     rhs=r16, start=True, stop=True)
                    oo_sb = opool.tile([D, QB], F16, tag="oo")
                    nc.vector.tensor_mul(oo_sb, oa_sb, bc[0:D, :])
                    nc.sync.dma_start(out=out_d[order[jidx]], in_=oo_sb)
                    return
                # the [65,512] PSUM->SBUF copy (fp32->fp16 in the op, DVE:
                # ACT is the exp-cadence-critical engine) moves numerator +
                # denominator together and frees the PSUM bank. For units
                # followed by a long slot it's split into column chunks and
                # pumped one per group boundary so the DVE exp chain never
                # takes a ~700ns hit in one group; the first two (short)
                # positions copy whole - their PSUM bank is re-needed a
                # single group later.
                oa_sb = gpool.tile([65, QB], F16, name=f"oa{jidx}",
                                   tag=f"oa{jidx}")
                # the batch-closing position copies whole so its
                # denominator row is ready before the batch Ln needs it
                nchunks = 1 if (jidx < 2 or jidx == rbatch[-1]) else 3
                step = -(-QB // nchunks)
                for c0 in range(0, QB, step):
                    c1 = min(QB, c0 + step)
                    if nchunks == 1:
                        nc.vector.tensor_copy(oa_sb[:, c0:c1], po[:, c0:c1])
                    else:
                        microq.append(
                            (0, lambda a=c0, b=c1:
                             nc.vector.tensor_copy(oa_sb[:, a:b],
                                                   po[:, a:b])))
                o_tiles[jidx] = oa_sb[0:D, :]
                if jidx in rbatch:
                    ri = rbatch.index(jidx)
                    if jidx == rbatch[-1]:
                        # immediate hop + near-term division: the whole
                        # chain (Ln/Exp/bounce/mults/out-DMAs) has ~5us of
                        # queue+DMA latency and must finish under the
                        # stream, not after it
                        nc.gpsimd.dma_start(out=dn_tile[ri:ri + 1, :],
                                            in_=oa_sb[64:65, :])
                        microq.append((cur["gi"] + 2, div_stage1))
                        for ui, jj in enumerate(rbatch):
                            microq.append((cur["gi"] + 4 + ui // 2,
                                           stage2_one(ui, jj)))
                    else:
                        microq.append(
                            (0, lambda: nc.gpsimd.dma_start(
                                out=dn_tile[ri:ri + 1, :],
                                in_=oa_sb[64:65, :])))
                else:
                    # second-to-last position: solo division via microq -
                    # its two ACT ops land in separate groups (each mostly
                    # absorbed by per-group ACT slack) instead of a 1.3us
                    # block that stalls the exp cadence; the broadcast is a
                    # single PE ones-matmul once r16 is ready.
                    g0 = cur["gi"]
                    lnd6 = gpool.tile([1, QB], F32, name="lnd6", tag="lnd6")
                    r166 = gpool.tile([1, QB], F16, name="r166", tag="r166")
                    microq.append(
                        (g0 + 2, lambda: nc.scalar.activation(
                            lnd6, oa_sb[64:65, :], AF.Ln)))
                    microq.append(
                        (g0 + 3, lambda: nc.scalar.activation(
                            r166, lnd6, AF.Exp, scale=-1.0)))

                    def fin6():
                        bc = psum_o.tile([65, QB], F32, tag="po")
                        nc.tensor.matmul(bc[0:D, :], lhsT=ones_sb,
                                         rhs=r166, start=True, stop=True)
                        oo_sb = opool.tile([D, QB], F16, tag="oo")
                        nc.vector.tensor_mul(oo_sb, o_tiles[jidx],
                                             bc[0:D, :])
                        nc.gpsimd.dma_start(out=out_d[order[jidx]],
                                            in_=oo_sb)
                    microq.append((g0 + 4, fin6))

            # all input DMAs are issued up front (spool/vpool hold one
            # buffer per slot, so no rotation waits). Q|K transfers lead
            # and V transfers trail two slots behind.
            open_qk(0)
            open_qk(1)
            open_qk(2)
            for jidx in range(3, NSLOTS):
                open_v(jidx - 3)
                open_qk(jidx)
            for jidx in range(NSLOTS - 3, NSLOTS):
                open_v(jidx)

            # flat k-tile schedule: exp groups are GK consecutive k-tiles
            # REGARDLESS of slot boundaries, so every exp op but the last
            # runs at the full N=1536 and slot transitions produce no
            # short-group hiccups
            flat = []
            for jidx, j in enumerate(order):
                w = slots_nk[j]
                for ki in range(w):
                    flat.append((jidx, ki, ki == w - 1))
            # group 0 covers slot 0 alone so the first exp gates only on
            # the first (smallest) QK transfer
            w0 = min(slots_nk[order[0]], GK)
            fgroups = [flat[:w0]] + [flat[i:i + GK]
                                     for i in range(w0, len(flat), GK)]

            def run_group(items, last=False):
                for pj, ki, closes, ph, p_prev in items:
                    _, pxv, ppo, pw = slot_ctx[pj]
                    pva = pxv.rearrange("p (w c) -> p w c", c=65)
                    nc.tensor.matmul(
                        ppo,
                        lhsT=pva[:, ki, :],
                        rhs=p_prev[:, ph * QB:(ph + 1) * QB],
                        start=(ki == 0), stop=(ki == pw - 1),
                    )
                    if closes:
                        close_slot(pj, last=last)

            def emit_exp(ps, p_sb, g, force_act=False):
                """Two-engine exp over a [128, g*512] PSUM group, split by
                query column (see module docstring). force_act runs the
                whole group on ACT - used for the first and last groups so
                the pipeline's ends don't wait on the DVE chain."""
                ww = g * QB
                if force_act:
                    nc.scalar.activation(p_sb[:, :ww], ps[:, :ww],
                                         AF.Exp, scale=0.125)
                    return
                ps3 = ps[:, :ww].rearrange("p (g q) -> p g q", q=QB)
                p3 = p_sb[:, :ww].rearrange("p (g q) -> p g q", q=QB)
                # ACT: table-exact exp on its column share
                nc.scalar.activation(
                    p3[:, :, 0:ACOL], ps3[:, :, 0:ACOL], AF.Exp, scale=0.125)
                # DVE: product bit-trick on its column share
                ia = tpool.tile([128, GK * TRICK], I16, tag="ia")
                ib = tpool.tile([128, GK * TRICK], I16, tag="ib")
                ia3 = ia[:, :g * TRICK].rearrange("p (g q) -> p g q", q=TRICK)
                ib3 = ib[:, :g * TRICK].rearrange("p (g q) -> p g q", q=TRICK)
                iaf = ia[:, :g * TRICK].bitcast(F16).rearrange(
                    "p (g q) -> p g q", q=TRICK)
                ibf = ib[:, :g * TRICK].bitcast(F16).rearrange(
                    "p (g q) -> p g q", q=TRICK)
                nc.vector.tensor_scalar(
                    ia3, ps3[:, :, ACOL:],
                    TS_SCALE, TS_BIAS, ALU.mult, ALU.add)
                nc.vector.tensor_scalar(ib3, ia3, 512.0, None, ALU.subtract)
                nc.vector.tensor_mul(p3[:, :, ACOL:], iaf, ibf)

            # While the PE HAM clock-gate is still cold (1.2 GHz), S
            # matmuls alone just keep pace with the exp stream, but S+O
            # would stall it: O-groups from the first COLD_GROUPS groups
            # defer entirely, then the backlog drains two-per-group down
            # to a steady O_LAG-group lag.
            ngroups = len(fgroups)
            oqueue = []   # deferred O-group batches, oldest first
            for gi, grp in enumerate(fgroups):
                cur["gi"] = gi
                g = len(grp)
                ps = psum_s.tile([128, GK * QB], F32, tag="ps")
                for i, (jidx, ki, _) in enumerate(grp):
                    qk_sb = slot_ctx[jidx][0]
                    qt_sb = qk_sb[:, 0:QB]
                    kt_sb = qk_sb[:, QB:]
                    rg = (ki % 2) * D   # row-group half = k-tile parity
                    nc.tensor.matmul(
                        ps[:, i * QB:(i + 1) * QB],
                        lhsT=kt_sb[rg:rg + D, (ki // 2) * KT:
                                   (ki // 2 + 1) * KT],
                        rhs=qt_sb[rg:rg + D, :],
                        start=True, stop=True,
                        tile_position=(rg, 0),
                    )
                if gi > COLD_GROUPS:
                    drains = (2 if len(oqueue) > O_LAG + 1 else
                              1 if len(oqueue) > O_LAG else 0)
                    for _ in range(drains):
                        run_group(oqueue.pop(0))
                p_sb = ppool.tile([128, GK * QB], F16, tag="p")
                emit_exp(ps, p_sb, g, force_act=(gi == 0))
                oqueue.append([(jidx, ki, closes, i, p_sb)
                               for i, (jidx, ki, closes) in enumerate(grp)])
                # pump harder near the end so no close work spills past
                # the last O-matmuls into the tail
                pump_micro(3 if gi >= ngroups - 6 else MICRO_PUMP)
            cur["gi"] = ngroups + 10
            pump_micro(drain=True)
            while oqueue:
                run_group(oqueue.pop(0), last=(len(oqueue) == 1))

    nc.compile()
    return nc


def _pack(queries, keys, values, valid_lens, slots_nk, offs, assign):
    order = _order(slots_nk)
    xw = [sum(_xw(w)) for w in slots_nk]
    tot = sum(xw)
    data = np.zeros((NCORES, 2 * D, tot), NPF16)
    for c in range(NCORES):
        x0 = 0
        for p, j in enumerate(order):
            b, qb = assign[c][j]
            w = slots_nk[j]
            wqk, wv = _xw(w)
            vl = int(valid_lens[b])
            blk = data[c, :, x0:x0 + xw[j]]
            qt = queries[b, qb * QB:(qb + 1) * QB, :].T      # [D, QB]
            blk[:D, 0:QB] = qt
            blk[D:, 0:QB] = qt
            # K^T tiles alternate partition halves: even k-tile i in rows
            # 0-63, odd in rows 64-127, both at column block i//2
            for i in range(w):
                half = (i % 2) * D
                c0 = QB + (i // 2) * KT
                blk[half:half + D, c0:c0 + KT] = (
                    keys[b, i * KT:(i + 1) * KT, :].T)
            vv = np.zeros((w * KT, 65), np.float32)
            vv[:vl, :D] = values[b, :vl, :]
            vv[:vl, D] = 1.0
            # [128 partitions, w, 65] flattened on the free axis
            blk[:, wqk:] = (
                vv.reshape(w, KT, 65).transpose(1, 0, 2).reshape(KT, w * 65))
            x0 += xw[j]
    return [{"data": data[c]} for c in range(NCORES)]


def kernel(queries, keys, values, valid_lens):
    global LAST_RESULTS
    queries = np.asarray(queries, dtype=np.float32)
    keys = np.asarray(keys, dtype=np.float32)
    values = np.asarray(values, dtype=np.float32)
    valid_lens = np.asarray(valid_lens)

    key = tuple(int(v) for v in valid_lens)
    if key not in _cache:
        nk, slots_nk, offs, assign = _schedule(valid_lens)
        nc = _build(slots_nk, offs)
        _cache[key] = (nc, slots_nk, offs, assign)
    nc, slots_nk, offs, assign = _cache[key]

    in_maps = _pack(queries, keys, values, valid_lens, slots_nk, offs, assign)
    res = run_bass_kernel_spmd(nc, in_maps, list(range(NCORES)))
    LAST_RESULTS = res

    out = np.empty((B, Q, D), np.float32)
    for c in range(NCORES):
        oc = res.results[c]["out"]          # [NSLOTS, D, QB]
        for j in range(NSLOTS):
            b, qb = assign[c][j]
            out[b, qb * QB:(qb + 1) * QB, :] = oc[j].T
    return out


# revision 69
# speedup vs baseline: 1.0195x; 1.0081x over previous
"""Masked dot-product attention (B=16, Q=K=2048, D=64) on 8 Trainium2 cores.

out = softmax(Q K^T / sqrt(64) + mask(valid_lens)) V, reproducing
reference.py's masked_softmax to ~7e-3 relative absmax (fp16 matmuls +
a bit-trick exp on two of the three exp engines, see below).

Sharding / load balance
-----------------------
Work units are (batch, 512-wide q-block): 64 units whose cost is
nk(b) = ceil(valid_len[b]/128) k-tiles. Units are sorted by nk descending and
dealt round-robin into 8 slots x 8 cores, so every core runs the *same*
static SPMD program (slot j processes NK_j = max-nk-of-its-rank-group
k-tiles) while the host packs each core's own data. Per-core inputs arrive
as packed [128, *] fp16 buffers per slot: [Q^T dup | K^T half-packed] and
[V_aug] (see _xw). Q|K and V ride separate DMAs, with V transfers trailing
so the early (ramp-limited) DMA bandwidth all feeds the S-matmul stream.

Device pipeline (inputs fp16; PSUM accumulates fp32)
----------------------------------------------------
  PE : S^T[128k, 512q] per k-tile = matmul(lhsT=K^T-tile, rhs=Q^T),
       contraction d=64, alternating k-tiles on PE row groups 0-63/64-127
  exp: P = exp(S^T/8) over the 3-bank PSUM group, split BY QUERY COLUMN
       across the two engines that can read PSUM (so each softmax row sees
       exactly one approximation and any per-engine scale cancels in the
       row's own denominator):
         - ACT cols [0:ACOL) of each 512 block: table-exact EXP ACTIVATE
         - DVE cols [ACOL:512): product bit-trick (below)
       (Pool/GPSIMD can't read PSUM and its tensor ops measured ~6-18x
       slower than modeled, so it only runs DMA issues here.)
  PE : O^T_aug[65, 512q] += matmul(lhsT=V_aug-tile[128,65], rhs=P-slice)

Product bit-trick exp (2 DVE-cycles/elem vs 1 ACT-cycle, but on otherwise
idle engines): exp(x/8) = 2^t (t = x*log2e/8) ~= f16bits(i) * f16bits(i-512)
where i = int16(x*64*log2e + BA). Each factor is a half-exponent Schraudolph
approximant 2^(t/2+c)*r(frac) with ripple r; the two factors sit exactly half
a mantissa-period apart, so the product's log-ripple cancels the odd
harmonics: +-1.5% max element error (vs +-6% for one trick). Empirically
7.3e-3 relative absmax end-to-end on the real inputs (tolerance 2e-2).
The int16 affine runs on fp32 PSUM directly; i-512 is an exact int
subtract; both factors multiply fp16->fp16 into the P tile.

O-matmuls lag one group behind S so the PE queue never head-of-line
blocks on exp. V_aug = [V | 1] with rows >= valid_len zeroed by the host
(exact masking, free denominator in row 64). A burst of dependency-free
tiny warm-up matmuls (reading the framework's pre-barrier const tile)
keeps the PE busy from t=0 until the first QK transfer lands, opening the
HAM clock gate to 2.4 GHz; the first COLD_GROUPS groups' O-matmuls defer
on top of that.

Close/division epilogue (all spread via a microtask FIFO)
---------------------------------------------------------
Per-unit close work is queued as thunks and pumped <=2-3 per group
boundary so no close ever inserts a multi-us bubble into an engine's exp
cadence: the [65,512] PSUM->SBUF numerator+denominator copy (DVE,
fp32->fp16) goes in 3 column chunks, the denominator-row hop rides the
Pool DMA queue. Reciprocals run on ACT as r = exp(-ln(d)) (Exp and Ln
share one pinned activation table set - no table switch): positions 0-5
as one batched [6,512] pair, broadcast across the 64 d-partitions via a
DRAM-bounce DMA (out-hop Sync / in-hop Pool overlap their SWDGE waits),
then fp16-multiplied and DMA'd out on alternating queues, all under the
stream. Position 6 runs the same chain solo with its two ACT ops in
separate groups and a ones-column PE broadcast. Only the final position
divides on the tail: Ln straight off its live PSUM denominator row, PE
broadcast, multiply, out. Output is fp16 [slot, 64, 512]; the host casts
and transposes O^T -> O while unsharding (numerators/denominators peak
~7.6e3 here, 8.6x under fp16 max).
"""

import sys

if "/opt/trn_rl_repo" not in sys.path:
    sys.path.insert(0, "/opt/trn_rl_repo")

import numpy as np

import bass_rust as _bass_rust
import concourse.bass as bass
import concourse.mybir as mybir
import concourse.tile as tile
from concourse import bacc
from concourse.bass_utils import run_bass_kernel_spmd

# NOTE: compiling with walrus --max-sem-num=176 (to shrink the ~6.5us
# fixed postamble that serially zeroes all 253 semaphores) passes the
# verifier but fails at runtime - the capped space collides with
# NRT-reserved high semaphores. Do not retry without a runtime fix.
from concourse.hw_specs import get_activation_tables

B, Q, KLEN, D = 16, 2048, 2048, 64
QB = 512                      # q-block width per work unit
NCORES = 8
NSLOTS = (B * (Q // QB)) // NCORES   # 8 slots per core
KT = 128                      # k-tile height
GK = 3                        # k-tiles per exp group (3 PSUM banks)
NWARM = 40                    # dependency-free tiny PE warm-up matmuls
                              # (~28ns each, opening the HAM clock gate;
                              # more would risk queueing ahead of the
                              # first real S-matmul on the in-order PE)
COLD_GROUPS = 4               # exp groups whose O-matmuls defer to warm PE
O_LAG = 1                     # groups the O-matmuls trail the exp stream by
F32 = mybir.dt.float32
F16 = mybir.dt.float16
I16 = mybir.dt.int16
NPF16 = np.float16
AF = mybir.ActivationFunctionType
ALU = mybir.AluOpType

# --- exp column split (per 512-wide block) ---
# Pool/GPSIMD cannot read PSUM and its int16 tensor ops run ~18x slower
# than the DVE (software Q7 path, measured 14.8ns/elem), so the trick
# share lives entirely on the DVE: affine PSUM->int16, int subtract,
# fp16 multiply, ~3.2ns/elem total vs ACT's exact-table 0.83ns/elem.
ACOL = 396                    # ACT table-exact exp
DVCOL = QB - ACOL             # DVE product bit-trick
TRICK = DVCOL
LOG2E = 1.4426950408889634
TS_SCALE = 64.0 * LOG2E       # i = round(S*TS_SCALE + TS_BIAS)
TS_BIAS = 15817.1             # 15*1024 - Ca, Ca = -457.1 (centering: cosmetic)

LAST_RESULTS = None           # BassKernelResults of the most recent run

_cache: dict = {}


class _Bacc(bacc.Bacc):
    """Bacc whose activation-table fixpoint keeps every ACTIVATE on one
    set: Exp and Ln both live in natural_log_exp_and_others, but the
    default selector binds Exp to exp_and_others and then pays two ~1.3us
    ACT_TABLE_LOAD switches around the division's Ln. Restricting Exp/Ln
    to the combined set yields a single load."""

    def insert_act_table_loads(self):
        has_activation = any(
            isinstance(i, mybir.InstActivation)
            for b in self.main_func.blocks
            for i in b.instructions
        )
        if not has_activation:
            return
        tables = []
        for name, fns in get_activation_tables(self.m.arch).items():
            if name != "natural_log_exp_and_others":
                fns = fns - {AF.Exp, AF.Ln}
            tables.append((name, fns))
        _bass_rust.insert_act_table_loads(self, tables)


def _schedule(valid_lens):
    """Static work schedule from valid_lens (host-known at call time)."""
    nk = [max(1, -(-int(v) // KT)) for v in valid_lens]
    units = [(b, qb) for b in range(B) for qb in range(Q // QB)]
    units.sort(key=lambda u: (-nk[u[0]], u))
    slots_nk = [nk[units[NCORES * j][0]] for j in range(NSLOTS)]
    assign = [[units[NCORES * j + c] for j in range(NSLOTS)] for c in range(NCORES)]
    offs = np.concatenate([[0], np.cumsum(slots_nk)]).tolist()
    return nk, slots_nk, offs, assign


def _order(slots_nk):
    """Processing order: two smallest first (their little DMAs land fast,
    so the exp stream starts early), then the rest descending so the
    serial input-DMA stream always runs ahead of compute, and the biggest
    slot last so division batches hide under the final slot's stream.
    (Both a strictly ascending variant and an ascending-through-position-4
    variant measured worse - the ~2.5-3us of early-group stalls are the
    aggregate DMA ramp, not any one transfer's position, and moving big
    slots later just shifts the stall into the stream's second half.)"""
    asc = sorted(range(NSLOTS), key=lambda j: (slots_nk[j], j))
    return [asc[0], asc[1], asc[2], asc[6], asc[4], asc[3], asc[5], asc[7]]


def _xw(w):
    """Per-slot packed widths: [Q^T dup | K^T half-packed] + [V_aug].

    K^T tiles alternate between partition halves (even k-tile i in rows
    0-63 at column block i//2, odd in rows 64-127) - half the K DMA bytes
    of a full duplication while keeping the PE row-group pairing; only
    the small Q^T block is duplicated. (Shipping Q once + an on-chip dup
    DMA measured WORSE: the extra issue serialization and the dup
    dependency on odd k-tiles cost more than the 65KB/slot of wire.)"""
    kk = (w + 1) // 2
    return QB + kk * KT, w * 65


def _build(slots_nk, offs):
    """Build + compile the single SPMD program for the given slot profile."""
    order = _order(slots_nk)
    xw = [sum(_xw(w)) for w in slots_nk]
    xoffs = np.concatenate([[0], np.cumsum([xw[j] for j in order])]).tolist()
    rbatch = [0, 1, 2, 3, 4, 5]   # one division batch: Ln/Exp cost is per
                                  # batch (free-dim cycles), not per unit

    nc = _Bacc()
    data_d = nc.dram_tensor("data", [2 * D, xoffs[-1]], F16,
                            kind="ExternalInput").ap()
    # fp16 output: the host casts to fp32 while unsharding; numerators and
    # denominators peak ~7.6e3 on this distribution, 8.6x under fp16 max
    out_d = nc.dram_tensor("out", [NSLOTS, D, QB], F16, kind="ExternalOutput").ap()

    with tile.TileContext(nc) as tc:
        with (
            tc.tile_pool(name="spool", bufs=8) as spool,
            tc.tile_pool(name="vpool", bufs=8) as vpool,
            tc.tile_pool(name="ppool", bufs=8) as ppool,
            tc.tile_pool(name="tpool", bufs=3) as tpool,
            tc.tile_pool(name="epool", bufs=3) as epool,
            tc.tile_pool(name="gpool", bufs=1) as gpool,
            tc.tile_pool(name="opool", bufs=8) as opool,
            tc.tile_pool(name="dpool", bufs=2, space="DRAM") as dpool,
            tc.tile_pool(name="psum_s", bufs=2, space="PSUM") as psum_s,
            tc.tile_pool(name="psum_o", bufs=2, space="PSUM") as psum_o,
        ):
            dn_tile = gpool.tile([len(rbatch), QB], F16, name="dn", tag="dn")
            ones_sb = gpool.tile([1, D], F16, name="ones", tag="ones")
            nc.vector.memset(ones_sb, 1.0)
            # first DMA warms the Sync HWDGE queue (its ~1.3us activation
            # latency) AND gives every DMA engine an early descriptor
            # ([128,128] spreads 16 descriptors round-robin), so the
            # per-engine rate ramp starts before the first real transfer
            wdma = gpool.tile([128, 128], F16, name="wdma", tag="wdma")
            nc.sync.dma_start(out=wdma, in_=data_d[:, 0:128])

            # PE warm-up: dependency-free matmuls from t=0 keep the PE busy
            # through the HAM activity window so the clock-gate opens to
            # 2.4 GHz before (or soon after) the first real matmul. They
            # read the framework's const tile (memset BEFORE the preamble
            # barrier) so the first one issues the moment the PE queue
            # reaches our block - no wait on our own memsets.
            cbf = nc.const_aps.aps[(mybir.dt.bfloat16, 1.0)]
            warm_ps = psum_o.tile([65, QB], F32, name="warm_ps", tag="po")
            for _ in range(NWARM):
                nc.tensor.matmul(warm_ps[0:1, 0:1], lhsT=cbf, rhs=cbf,
                                 start=True, stop=True)

            o_tiles = {}
            # Per-unit close work (PSUM->SBUF copy chunks, denominator-row
            # hops, the division batch, final multiplies) is spread across
            # group boundaries via this FIFO of thunks: at most MICRO_PUMP
            # of them run per boundary, so no single close ever inserts a
            # multi-us bubble into the DVE/ACT exp cadence.
            microq = []      # (min_gi, fn): fn emits once cur group >= min_gi
            cur = {"gi": 0}
            MICRO_PUMP = 2

            def pump_micro(n=MICRO_PUMP, drain=False):
                ran = 0
                i = 0
                while i < len(microq) and (ran < n or drain):
                    if drain or microq[i][0] <= cur["gi"]:
                        microq.pop(i)[1]()
                        ran += 1
                    else:
                        i += 1

            rb_holder = {}

            def div_stage1():
                # r = exp(-ln(d)) on ACT, one batched [6,512] op pair
                # (free-dim cycles are paid once per batch); both functions
                # share the pinned activation table set, so no table
                # switch. Broadcast across the 64 d-partitions via a DRAM
                # bounce (DRAM is flat, so the read-back replicates with a
                # stride-0 leading dim). Both hops ride the Pool queue so
                # they never block Sync input issues.
                n = len(rbatch)
                lnd = epool.tile([n, QB], F32, tag="lnd")
                nc.scalar.activation(lnd, dn_tile, AF.Ln)
                r_sb = epool.tile([n, QB], F16, tag="r")
                nc.scalar.activation(r_sb, lnd, AF.Exp, scale=-1.0)
                scratch = dpool.tile([n, QB], F16, tag="scr")
                # out-hop on Sync (idle mid-stream): overlapping the two
                # hops across queues shaves the serialized SWDGE waits
                nc.sync.dma_start(out=scratch, in_=r_sb)
                rb_sb = epool.tile([D, n, QB], F16, tag="rb")
                bcast_src = bass.AP(
                    tensor=scratch.tensor,
                    offset=scratch.offset,
                    ap=[[0, D]] + [list(a) for a in scratch.ap],
                )
                nc.gpsimd.dma_start(out=rb_sb, in_=bcast_src)
                rb_holder["rb"] = rb_sb

            def stage2_one(ui, jj):
                def f():
                    oo_sb = opool.tile([D, QB], F16, tag="oo")
                    nc.vector.tensor_mul(oo_sb, o_tiles[jj],
                                         rb_holder["rb"][:, ui, :])
                    # alternate queues: 8 back-to-back issues on one queue
                    # serialize ~5us of tail otherwise
                    eng = nc.sync if jj % 2 == 0 else nc.gpsimd
                    eng.dma_start(out=out_d[order[jj]], in_=oo_sb)
                return f

            slot_ctx = {}

            def open_qk(jidx):
                j = order[jidx]
                w = slots_nk[j]
                wqk, wv = _xw(w)
                qk_sb = spool.tile([2 * D, wqk], F16, tag="xqk")
                # early QK issues alternate Sync / ACT-HWDGE: the ACT queue
                # is idle until its first exp (~10.7us), so pairing the
                # issues halves the ~0.6us-each serialization right where
                # the stream is input-supply-limited
                eng = nc.scalar if jidx in (1, 3) else nc.sync
                eng.dma_start(
                    out=qk_sb,
                    in_=data_d[:, xoffs[jidx]:xoffs[jidx] + wqk])
                po = psum_o.tile([65, QB], F32, tag="po")
                slot_ctx[jidx] = [qk_sb, None, po, w]

            def open_v(jidx):
                j = order[jidx]
                w = slots_nk[j]
                wqk, wv = _xw(w)
                xv_sb = vpool.tile([2 * D, wv], F16, tag="xv")
                nc.sync.dma_start(
                    out=xv_sb,
                    in_=data_d[:, xoffs[jidx] + wqk:xoffs[jidx] + wqk + wv])
                slot_ctx[jidx][1] = xv_sb

            def close_slot(jidx, last=False):
                _, _, po, _ = slot_ctx[jidx]
                if jidx == NSLOTS - 1:
                    # final position: the whole division chain runs inline
                    # at the close - Ln straight off the PSUM denominator
                    # row in parallel with the DVE numerator copy, then
                    # r = exp(-ln(d)), ones-column PE broadcast, multiply,
                    # out. All tiles are dedicated (gpool) so no pool-
                    # rotation dependency can delay this tail.
                    lnd = gpool.tile([1, QB], F32, name=f"lnd{jidx}",
                                     tag=f"lnd{jidx}")
                    nc.scalar.activation(lnd, po[64:65, :], AF.Ln)
                    oa_sb = gpool.tile([D, QB], F16, name=f"oa{jidx}",
                                       tag=f"oa{jidx}")
                    nc.vector.tensor_copy(oa_sb, po[0:64, :])
                    r16 = gpool.tile([1, QB], F16, name=f"r16{jidx}",
                                     tag=f"r16{jidx}")
                    nc.scalar.activation(r16, lnd, AF.Exp, scale=-1.0)
                    # broadcast target from the psum_o pool: the rotation
                    # lands it on this unit's own accumulator bank, whose
                    # readers are this chain's own upstream
                    bc = psum_o.tile([65, QB], F32, tag="po")
                    nc.tensor.matmul(bc[0:D, :], lhsT=ones_sb,
                                     rhs=r16, start=True, stop=True)
                    oo_sb = opool.tile([D, QB], F16, tag="oo")
                    nc.vector.tensor_mul(oo_sb, oa_sb, bc[0:D, :])
                    nc.sync.dma_start(out=out_d[order[jidx]], in_=oo_sb)
                    return
                # the [65,512] PSUM->SBUF copy (fp32->fp16 in the op, DVE:
                # ACT is the exp-cadence-critical engine) moves numerator +
                # denominator together and frees the PSUM bank. For units
                # followed by a long slot it's split into column chunks and
                # pumped one per group boundary so the DVE exp chain never
                # takes a ~700ns hit in one group; the first two (short)
                # positions copy whole - their PSUM bank is re-needed a
                # single group later.
                oa_sb = gpool.tile([65, QB], F16, name=f"oa{jidx}",
                                   tag=f"oa{jidx}")
                # the batch-closing position copies whole so its
                # denominator row is ready before the batch Ln needs it
                nchunks = 1 if (jidx < 2 or jidx == rbatch[-1]) else 3
                step = -(-QB // nchunks)
                for c0 in range(0, QB, step):
                    c1 = min(QB, c0 + step)
                    if nchunks == 1:
                        nc.vector.tensor_copy(oa_sb[:, c0:c1], po[:, c0:c1])
                    else:
                        microq.append(
                            (0, lambda a=c0, b=c1:
                             nc.vector.tensor_copy(oa_sb[:, a:b],
                                                   po[:, a:b])))
                o_tiles[jidx] = oa_sb[0:D, :]
                if jidx in rbatch:
                    ri = rbatch.index(jidx)
                    if jidx == rbatch[-1]:
                        # immediate hop + near-term division: the whole
                        # chain (Ln/Exp/bounce/mults/out-DMAs) has ~5us of
                        # queue+DMA latency and must finish under the
                        # stream, not after it
                        nc.gpsimd.dma_start(out=dn_tile[ri:ri + 1, :],
                                            in_=oa_sb[64:65, :])
                        microq.append((cur["gi"] + 2, div_stage1))
                        for ui, jj in enumerate(rbatch):
                            microq.append((cur["gi"] + 4 + ui // 2,
                                           stage2_one(ui, jj)))
                    else:
                        microq.append(
                            (0, lambda: nc.gpsimd.dma_start(
                                out=dn_tile[ri:ri + 1, :],
                                in_=oa_sb[64:65, :])))
                else:
                    # second-to-last position: solo division via microq -
                    # its two ACT ops land in separate groups (each mostly
                    # absorbed by per-group ACT slack) instead of a 1.3us
                    # block that stalls the exp cadence; the broadcast is a
                    # single PE ones-matmul once r16 is ready.
                    g0 = cur["gi"]
                    lnd6 = gpool.tile([1, QB], F32, name="lnd6", tag="lnd6")
                    r166 = gpool.tile([1, QB], F16, name="r166", tag="r166")
                    microq.append(
                        (g0 + 2, lambda: nc.scalar.activation(
                            lnd6, oa_sb[64:65, :], AF.Ln)))
                    microq.append(
                        (g0 + 3, lambda: nc.scalar.activation(
                            r166, lnd6, AF.Exp, scale=-1.0)))

                    def fin6():
                        bc = psum_o.tile([65, QB], F32, tag="po")
                        nc.tensor.matmul(bc[0:D, :], lhsT=ones_sb,
                                         rhs=r166, start=True, stop=True)
                        oo_sb = opool.tile([D, QB], F16, tag="oo")
                        nc.vector.tensor_mul(oo_sb, o_tiles[jidx],
                                             bc[0:D, :])
                        nc.gpsimd.dma_start(out=out_d[order[jidx]],
                                            in_=oo_sb)
                    microq.append((g0 + 4, fin6))

            # all input DMAs are issued up front (spool/vpool hold one
            # buffer per slot, so no rotation waits). Q|K transfers lead
            # and V transfers trail two slots behind.
            open_qk(0)
            open_qk(1)
            open_qk(2)
            for jidx in range(3, NSLOTS):
                open_v(jidx - 3)
                open_qk(jidx)
            for jidx in range(NSLOTS - 3, NSLOTS):
                open_v(jidx)

            # flat k-tile schedule: exp groups are GK consecutive k-tiles
            # REGARDLESS of slot boundaries, so every exp op but the last
            # runs at the full N=1536 and slot transitions produce no
            # short-group hiccups
            flat = []
            for jidx, j in enumerate(order):
                w = slots_nk[j]
                for ki in range(w):
                    flat.append((jidx, ki, ki == w - 1))
            # group 0 covers slot 0 alone so the first exp gates only on
            # the first (smallest) QK transfer
            w0 = min(slots_nk[order[0]], GK)
            fgroups = [flat[:w0]] + [flat[i:i + GK]
                                     for i in range(w0, len(flat), GK)]

            def run_group(items, last=False):
                for pj, ki, closes, ph, p_prev in items:
                    _, pxv, ppo, pw = slot_ctx[pj]
                    pva = pxv.rearrange("p (w c) -> p w c", c=65)
                    nc.tensor.matmul(
                        ppo,
                        lhsT=pva[:, ki, :],
                        rhs=p_prev[:, ph * QB:(ph + 1) * QB],
                        start=(ki == 0), stop=(ki == pw - 1),
                    )
                    if closes:
                        close_slot(pj, last=last)

            def emit_exp(ps, p_sb, g, force_act=False):
                """Two-engine exp over a [128, g*512] PSUM group, split by
                query column (see module docstring). force_act runs the
                whole group on ACT - used for the first and last groups so
                the pipeline's ends don't wait on the DVE chain."""
                ww = g * QB
                if force_act:
                    nc.scalar.activation(p_sb[:, :ww], ps[:, :ww],
                                         AF.Exp, scale=0.125)
                    return
                ps3 = ps[:, :ww].rearrange("p (g q) -> p g q", q=QB)
                p3 = p_sb[:, :ww].rearrange("p (g q) -> p g q", q=QB)
                # ACT: table-exact exp on its column share
                nc.scalar.activation(
                    p3[:, :, 0:ACOL], ps3[:, :, 0:ACOL], AF.Exp, scale=0.125)
                # DVE: product bit-trick on its column share
                ia = tpool.tile([128, GK * TRICK], I16, tag="ia")
                ib = tpool.tile([128, GK * TRICK], I16, tag="ib")
                ia3 = ia[:, :g * TRICK].rearrange("p (g q) -> p g q", q=TRICK)
                ib3 = ib[:, :g * TRICK].rearrange("p (g q) -> p g q", q=TRICK)
                iaf = ia[:, :g * TRICK].bitcast(F16).rearrange(
                    "p (g q) -> p g q", q=TRICK)
                ibf = ib[:, :g * TRICK].bitcast(F16).rearrange(
                    "p (g q) -> p g q", q=TRICK)
                nc.vector.tensor_scalar(
                    ia3, ps3[:, :, ACOL:],
                    TS_SCALE, TS_BIAS, ALU.mult, ALU.add)
                nc.vector.tensor_scalar(ib3, ia3, 512.0, None, ALU.subtract)
                nc.vector.tensor_mul(p3[:, :, ACOL:], iaf, ibf)

            # While the PE HAM clock-gate is still cold (1.2 GHz), S
            # matmuls alone just keep pace with the exp stream, but S+O
            # would stall it: O-groups from the first COLD_GROUPS groups
            # defer entirely, then the backlog drains two-per-group down
            # to a steady O_LAG-group lag.
            ngroups = len(fgroups)
            oqueue = []   # deferred O-group batches, oldest first
            for gi, grp in enumerate(fgroups):
                cur["gi"] = gi
                g = len(grp)
                ps = psum_s.tile([128, GK * QB], F32, tag="ps")
                for i, (jidx, ki, _) in enumerate(grp):
                    qk_sb = slot_ctx[jidx][0]
                    qt_sb = qk_sb[:, 0:QB]
                    kt_sb = qk_sb[:, QB:]
                    rg = (ki % 2) * D   # row-group half = k-tile parity
                    nc.tensor.matmul(
                        ps[:, i * QB:(i + 1) * QB],
                        lhsT=kt_sb[rg:rg + D, (ki // 2) * KT:
                                   (ki // 2 + 1) * KT],
                        rhs=qt_sb[rg:rg + D, :],
                        start=True, stop=True,
                        tile_position=(rg, 0),
                    )
                if gi > COLD_GROUPS:
                    drains = (2 if len(oqueue) > O_LAG + 1 else
                              1 if len(oqueue) > O_LAG else 0)
                    for _ in range(drains):
                        run_group(oqueue.pop(0))
                p_sb = ppool.tile([128, GK * QB], F16, tag="p")
                emit_exp(ps, p_sb, g, force_act=(gi == 0))
                oqueue.append([(jidx, ki, closes, i, p_sb)
                               for i, (jidx, ki, closes) in enumerate(grp)])
                # pump harder near the end so no close work spills past
                # the last O-matmuls into the tail
                pump_micro(3 if gi >= ngroups - 6 else MICRO_PUMP)
            cur["gi"] = ngroups + 10
            pump_micro(drain=True)
            while oqueue:
                run_group(oqueue.pop(0), last=(len(oqueue) == 1))

    nc.compile()
    return nc


def _pack(queries, keys, values, valid_lens, slots_nk, offs, assign):
    order = _order(slots_nk)
    xw = [sum(_xw(w)) for w in slots_nk]
    tot = sum(xw)
    data = np.zeros((NCORES, 2 * D, tot), NPF16)
    for c in range(NCORES):
        x0 = 0
        for p, j in enumerate(order):
            b, qb = assign[c][j]
            w = slots_nk[j]
            wqk, wv = _xw(w)
            vl = int(valid_lens[b])
            blk = data[c, :, x0:x0 + xw[j]]
            qt = queries[b, qb * QB:(qb + 1) * QB, :].T      # [D, QB]
            blk[:D, 0:QB] = qt
            blk[D:, 0:QB] = qt
            # K^T tiles alternate partition halves: even k-tile i in rows
            # 0-63, odd in rows 64-127, both at column block i//2
            for i in range(w):
                half = (i % 2) * D
                c0 = QB + (i // 2) * KT
                blk[half:half + D, c0:c0 + KT] = (
                    keys[b, i * KT:(i + 1) * KT, :].T)
            vv = np.zeros((w * KT, 65), np.float32)
            vv[:vl, :D] = values[b, :vl, :]
            vv[:vl, D] = 1.0
            # [128 partitions, w, 65] flattened on the free axis
            blk[:, wqk:] = (
                vv.reshape(w, KT, 65).transpose(1, 0, 2).reshape(KT, w * 65))
            x0 += xw[j]
    return [{"data": data[c]} for c in range(NCORES)]


def kernel(queries, keys, values, valid_lens):
    global LAST_RESULTS
    queries = np.asarray(queries, dtype=np.float32)
    keys = np.asarray(keys, dtype=np.float32)
    values = np.asarray(values, dtype=np.float32)
    valid_lens = np.asarray(valid_lens)

    key = tuple(int(v) for v in valid_lens)
    if key not in _cache:
        nk, slots_nk, offs, assign = _schedule(valid_lens)
        nc = _build(slots_nk, offs)
        _cache[key] = (nc, slots_nk, offs, assign)
    nc, slots_nk, offs, assign = _cache[key]

    in_maps = _pack(queries, keys, values, valid_lens, slots_nk, offs, assign)
    res = run_bass_kernel_spmd(nc, in_maps, list(range(NCORES)))
    LAST_RESULTS = res

    out = np.empty((B, Q, D), np.float32)
    for c in range(NCORES):
        oc = res.results[c]["out"]          # [NSLOTS, D, QB]
        for j in range(NSLOTS):
            b, qb = assign[c][j]
            out[b, qb * QB:(qb + 1) * QB, :] = oc[j].T
    return out
